# revision 25
# baseline (speedup 1.0000x reference)
"""v5: v4 + head/tail overhaul from trace analysis.

Trace findings on v4 (189.4us):
- scalar (ACT) is the pacer: 100% busy 38us..157us; head has ~14us of
  ACT idle (koh DMA queued behind 2MB of kt tiles -> 8.4us full-pipe
  stall), tail has ~13us after the last exp (fp32 LOW_HIGH ones-fold
  matmuls ~8.5us + serialized drains/DMAs), plus ~16us fixed NEFF
  pre/postamble.

v5 changes:
- DMA order: kq0 split (first 128 cols land first), kq1, koh_a (first
  16 jj), kq2, kq3, klab, qlabb, kts1, koh_b, kts2-4, qoh (bf16).
  Everything lands >=1us before its first consumer.
- LAG taper: 8 for jj<12 shrinking to 2 by jj=18, so the first oh
  matmul enters the PE queue only after koh_a has landed; exp pool
  widened to 10 bufs.
- init_S segmented: one zoh matmul after do_jj(2,4,6,8) instead of a
  2us block that starved ACT at startup.
- rowacc2 [128, 2*(NJJ-1)]: DVE cache-reduce accum -> col 2jj, ACT
  activation accum -> col 2jj+1, memset once, host sums the pair.
  Removes 79 TENSOR_REDUCEs and the racc indirection.
- folds in bf16 inside the steady state: per drained bank, S bank ->
  bf16 sc_b, pm_b = sc_b * qoh (TT bf16 2x), two bf16 ones-matmuls
  (aq/pq) into S rows 0/1 of the dead bank, [2,512] copy to SBUF.
  Replaces the tail's fp32 folds entirely.
- output DMAs chunked/early: rowacc2 cols for jj<64 DMA'd mid-kernel.
"""

import sys

if "/opt/trn_rl_repo" not in sys.path:
    sys.path.insert(0, "/opt/trn_rl_repo")

from collections import deque

import numpy as np
import ml_dtypes

N = 16384
D = 128
NC = 8
RPC = N // NC
QB = RPC // 128
NBLK = N // 128
NJJ = 80
TEMP = 0.5
BF16 = ml_dtypes.bfloat16

# Schraudolph exp in bf16 bits: bits = round(x*(128/ln2) + 128*(127-c)).
# x = psum/TEMP, folded into the multiplier. Row-sum ratio error ~0.1%
# and the ~+0.3% mean bias cancels between pos_sum and all_sum.
SCH_A = (1.0 / TEMP) * 128.0 / float(np.log(2.0))
SCH_B = 128.0 * (127.0 - 0.0436)

_prog_cache = {}


def _seg512(a, b):
    """Split [a, b) at absolute multiples of 512 (matmul ISA max width)."""
    out = []
    while a < b:
        n = min((a // 512 + 1) * 512, b) - a
        out.append((a, a + n))
        a += n
    return out


def _build_program(dmax):
    import concourse.bacc as bacc
    import concourse.tile as tile
    import concourse.mybir as mybir

    dt = mybir.dt
    AF = mybir.ActivationFunctionType
    ALU = mybir.AluOpType

    PW = min(NJJ - 1, QB + dmax)
    KOH_A = 16 * 64  # one-hot cols for jj 0..15

    nc = bacc.Bacc(
        "TRN2",
        target_bir_lowering=False,
        debug=False,
        enable_asserts=False,
        num_devices=NC,
    )

    kt_d = nc.dram_tensor("kt", [D, N], dt.bfloat16, kind="ExternalInput").ap()
    koh_d = nc.dram_tensor("koh", [128, NJJ * 64], dt.bfloat16, kind="ExternalInput").ap()
    klab_d = nc.dram_tensor("klab", [128, NJJ], dt.float32, kind="ExternalInput").ap()
    qlabb_d = nc.dram_tensor("qlabb", [128, RPC], dt.bfloat16, kind="ExternalInput").ap()
    qoh_d = nc.dram_tensor("qoh", [128, RPC], dt.bfloat16, kind="ExternalInput").ap()

    rowacc_d = nc.dram_tensor(
        "rowacc2", [128, 2 * (NJJ - 1)], dt.float32, kind="ExternalOutput").ap()
    poskey_d = nc.dram_tensor("poskey", [128, PW], dt.float32, kind="ExternalOutput").ap()
    aqpq_d = nc.dram_tensor("aqpq", [2, RPC], dt.float32, kind="ExternalOutput").ap()

    with tile.TileContext(nc) as tc:
        with (
            tc.tile_pool(name="keys", bufs=1) as keys_pool,
            tc.tile_pool(name="aux", bufs=1) as aux_pool,
            tc.tile_pool(name="ps", bufs=2, space="PSUM") as psum_pool,
            tc.tile_pool(name="sacc", bufs=1, space="PSUM") as sacc_pool,
            tc.tile_pool(name="ex", bufs=10) as exp_pool,
            tc.tile_pool(name="jk", bufs=2) as junk_pool,
            tc.tile_pool(name="fin", bufs=1) as fin_pool,
        ):
            # --- input DMAs, deadline-ordered on the sync HWDGE ring ---
            kq = keys_pool.tile([D, 2048], dt.bfloat16, tag="kq", name="kq")
            koh_s = aux_pool.tile([128, NJJ * 64], dt.bfloat16, tag="koh")
            klab_s = aux_pool.tile([128, NJJ], dt.float32, tag="klab")
            qlabb_s = aux_pool.tile([128, RPC], dt.bfloat16, tag="qlabb")
            qoh_s = aux_pool.tile([128, RPC], dt.bfloat16, tag="qoh")
            kts = [None] + [
                keys_pool.tile([D, 2048], dt.bfloat16, tag=f"kt{j}", name=f"kt{j}")
                for j in range(1, 5)]

            nc.sync.dma_start(kq[:, 0:128], kt_d[:, 0:128])
            nc.sync.dma_start(kq[:, 128:512], kt_d[:, 128:512])
            nc.sync.dma_start(kq[:, 512:1024], kt_d[:, 512:1024])
            nc.sync.dma_start(koh_s[:, 0:KOH_A], koh_d[:, 0:KOH_A])
            nc.sync.dma_start(kq[:, 1024:1536], kt_d[:, 1024:1536])
            nc.sync.dma_start(kq[:, 1536:2048], kt_d[:, 1536:2048])
            nc.sync.dma_start(klab_s[:], klab_d[:])
            nc.sync.dma_start(qlabb_s[:], qlabb_d[:])
            nc.sync.dma_start(kts[1][:], kt_d[:, 2048:4096])
            nc.sync.dma_start(koh_s[:, KOH_A:], koh_d[:, KOH_A:])
            nc.sync.dma_start(kts[2][:], kt_d[:, 4096:6144])
            nc.sync.dma_start(kts[3][:], kt_d[:, 6144:8192])
            nc.sync.dma_start(kts[4][:], kt_d[:, 8192:10240])
            nc.sync.dma_start(qoh_s[:], qoh_d[:])

            def kt_block(b):  # [128, 128] lhsT slice for key block b
                col = b * 128
                if col < 2048:
                    return kq[:, col:col + 128]
                return kts[col // 2048][:, col % 2048:col % 2048 + 128]

            zoh = aux_pool.tile([128, 128], dt.bfloat16, tag="zoh")
            nc.vector.memset(zoh[:], 0.0)
            ones128 = aux_pool.tile([128, 1], dt.bfloat16, tag="ones128")
            nc.vector.memset(ones128[:], 1.0)

            rowacc = fin_pool.tile([128, 2 * (NJJ - 1)], dt.float32, tag="rowacc")
            nc.vector.memset(rowacc[:], 0.0)
            poskey = fin_pool.tile([128, PW], dt.float32, tag="poskey")

            # doubled S accumulator: rows 0-63 and 64-127 are independent
            # halves (col-group tiling); folded per-bank by ones-matmuls.
            S = sacc_pool.tile([128, RPC], dt.float32, tag="sacc")

            def init_S_seg(t):
                nc.tensor.matmul(
                    S[:, t * 512:(t + 1) * 512], zoh[:],
                    kq[:, 0:512],
                    start=True, stop=False, skip_group_check=True,
                )

            oh_ctr = [0]

            def oh_mm(jj, lo, a, b, ex, stop):
                for (s0, s1) in _seg512(a, b):
                    th = oh_ctr[0] % 2
                    oh_ctr[0] += 1
                    nc.tensor.matmul(
                        S[th * 64:(th + 1) * 64, s0:s1],
                        koh_s[:, jj * 64:(jj + 1) * 64],
                        ex[:, s0 - lo:s1 - lo],
                        start=False, stop=stop,
                        skip_group_check=True,
                        tile_position=(0, th * 64),
                    )

            pending = deque()

            def mode_for(jj):
                # 'b': exact, window-1 rowsum on DVE cache-reduce (early,
                #      poskey band jj; also keeps the diagonal exact)
                # 'A': window-2 exp on DVE (Schraudolph bits; window-1 stays
                #      on ACT with accum so the psum ring never waits on the
                #      DVE), window-2 rowsum via DVE cache-reduce
                # 'a': window-1 exp on DVE (for jj>=64 whose window-1 holds
                #      the extra region and can't use the ACT accum)
                # 'c': exact, BOTH windows' rowsums on ACT accumulators;
                #      frees ~1200ns of DVE per jj
                if jj < 10:
                    return 'b'
                if jj < 20:
                    # ramp: window-2 offload keeps the ACT fed while DMAs
                    # stream; the approximate diagonal is subtracted exactly
                    # on the host and its ~3% ripple is negligible vs the sums
                    return 'A'
                if jj >= 72:
                    # single-window tail jj: offload would serialize on DVE,
                    # 'c' would pollute rowacc with the extra region
                    return 'b'
                if jj >= 64:
                    return 'a'
                return 'A' if jj % 3 != 2 else 'c'

            def service(rec):
                jj, lo, end, acc_lo, oh_end, stt_lo, ex, mode = rec
                # one-hot class sums: extra (d=64) part full weight, main
                # part excludes the d=0 block
                if lo < acc_lo:
                    oh_mm(jj, lo, lo, acc_lo, ex, stop=True)
                if acc_lo < oh_end:
                    oh_mm(jj, lo, acc_lo, oh_end, ex, stop=False)
                # per-key row sum, first window's share on the DVE (the
                # second window accumulated on ACT during exp)
                d1 = 1024 if (lo < 1024 and end > 1024) else end
                if mode == 'A':
                    # window-2 was Schraudolph'd on the DVE: its rowsum too
                    junk = junk_pool.tile([128, 2048], dt.bfloat16, tag="jk", name="jk")
                    nc.vector.tensor_scalar(
                        junk[:, :end - d1], ex[:, d1 - lo:end - lo],
                        1.0, 0.0, ALU.mult, ALU.add,
                        accum_out=rowacc[:, 2 * jj + 1:2 * jj + 2],
                    )
                elif mode != 'c' and acc_lo < d1:
                    junk = junk_pool.tile([128, 2048], dt.bfloat16, tag="jk", name="jk")
                    nc.vector.tensor_scalar(
                        junk[:, :d1 - acc_lo], ex[:, acc_lo - lo:d1 - lo],
                        1.0, 0.0, ALU.mult, ALU.add,
                        accum_out=rowacc[:, 2 * jj:2 * jj + 1],
                    )
                # per-key positive sum over the same-class band
                if stt_lo is not None and stt_lo < end:
                    mk = junk_pool.tile([128, 2048], dt.bfloat16, tag="jk", name="mk")
                    nc.vector.scalar_tensor_tensor(
                        mk[:, :end - stt_lo], qlabb_s[:, stt_lo:end],
                        klab_s[:, jj:jj + 1],
                        ex[:, stt_lo - lo:end - lo],
                        ALU.is_equal, ALU.mult,
                        accum_out=poskey[:, jj:jj + 1],
                    )

            def lag_for(jj):
                return max(2, min(8, 8 - (jj - 11)))

            def do_jj(jj):
                main_lo = max(0, jj - 63)
                hi = min(QB - 1, jj)
                lo = (jj - 64 if jj >= 64 else main_lo) * 128
                end = (hi + 1) * 128
                acc_lo = main_lo * 128
                oh_end = min(end, jj * 128) if jj <= QB - 1 else end
                stt_lo = max(acc_lo, (jj - dmax) * 128) if jj <= QB - 1 + dmax else None

                mode = mode_for(jj)
                ex = exp_pool.tile([128, 2048], dt.bfloat16, tag="ex", name="ex")
                k = 0
                w0 = (lo // 1024) * 1024
                while w0 < end:
                    p_lo = max(w0, lo)
                    p_end = min(w0 + 1024, end)
                    if p_lo < p_end:
                        ps = psum_pool.tile([128, 1024], dt.float32, tag="ps", name="ps")
                        for (s0, s1) in _seg512(p_lo, p_end):
                            nc.tensor.matmul(
                                ps[:, s0 - w0:s1 - w0],
                                kt_block(jj), kq[:, s0:s1],
                            )
                        if k == 1 and mode == 'A':
                            # Schraudolph window-2 on the DVE: window-1's
                            # psum consumer stays the (fast) ACT
                            nc.vector.tensor_scalar(
                                ex[:, p_lo - lo:p_end - lo].bitcast(dt.int16),
                                ps[:, p_lo - w0:p_end - w0],
                                SCH_A, SCH_B, ALU.mult, ALU.add,
                            )
                        elif k == 1 and p_lo >= acc_lo:
                            # second window: row-sum for free on the ACT accum
                            nc.scalar.activation(
                                ex[:, p_lo - lo:p_end - lo],
                                ps[:, p_lo - w0:p_end - w0],
                                AF.Exp, scale=1.0 / TEMP,
                                accum_out=rowacc[:, 2 * jj + 1:2 * jj + 2],
                            )
                        elif k == 0 and mode == 'A':
                            # first window: row-sum on the ACT accum
                            nc.scalar.activation(
                                ex[:, p_lo - lo:p_end - lo],
                                ps[:, p_lo - w0:p_end - w0],
                                AF.Exp, scale=1.0 / TEMP,
                                accum_out=rowacc[:, 2 * jj:2 * jj + 1],
                            )
                        elif k == 0 and mode == 'a':
                            # Schraudolph: bf16 bits via DVE fma + int16
                            # round, frees the ACT for other windows
                            nc.vector.tensor_scalar(
                                ex[:, p_lo - lo:p_end - lo].bitcast(dt.int16),
                                ps[:, p_lo - w0:p_end - w0],
                                SCH_A, SCH_B, ALU.mult, ALU.add,
                            )
                        elif k == 0 and mode == 'c':
                            # first window: row-sum on the ACT accum too
                            nc.scalar.activation(
                                ex[:, p_lo - lo:p_end - lo],
                                ps[:, p_lo - w0:p_end - w0],
                                AF.Exp, scale=1.0 / TEMP,
                                accum_out=rowacc[:, 2 * jj:2 * jj + 1],
                            )
                        else:
                            nc.scalar.activation(
                                ex[:, p_lo - lo:p_end - lo],
                                ps[:, p_lo - w0:p_end - w0],
                                AF.Exp, scale=1.0 / TEMP,
                            )
                        k += 1
                    w0 += 1024
                pending.append((jj, lo, end, acc_lo, oh_end, stt_lo, ex, mode))
                while len(pending) > lag_for(jj):
                    service(pending.popleft())

            aqpq_s = fin_pool.tile([33, RPC], dt.float32, tag="aqpqs")

            def fold_bank(b):
                sl = slice(b * 512, (b + 1) * 512)
                sc_b = fin_pool.tile([128, 512], dt.bfloat16, tag=f"sc{b}")
                nc.vector.tensor_copy(sc_b[:], S[:, sl])
                pm_b = fin_pool.tile([128, 512], dt.bfloat16, tag=f"pm{b}")
                nc.vector.tensor_mul(pm_b[:], sc_b[:], qoh_s[:, sl])
                # fold halves+classes via bf16 ones-matmuls into the dead
                # bank's rows 0 (all-sum) and 32 (pos-sum; matmul output
                # base partition must be 0/32/64)
                nc.tensor.matmul(
                    S[0:1, sl], ones128[:], sc_b[:],
                    start=True, stop=True, skip_group_check=True)
                nc.tensor.matmul(
                    S[32:33, sl], ones128[:], pm_b[:],
                    start=True, stop=True, skip_group_check=True)
                # one copy spanning rows 0..32 costs the same as [1,512]
                # (free-dim bound); rows 1..31 are don't-care
                nc.vector.tensor_copy(aqpq_s[:, sl], S[0:33, sl])

            do_jj(0)
            for jj in range(1, NJJ):
                do_jj(jj)
                if jj in (2, 4, 6, 8):
                    init_S_seg((jj - 2) // 2)
                # S bank b (query cols [b*512,(b+1)*512)) is final after
                # oh(67+4b), serviced during do_jj(69+4b): fold right away,
                # hidden under the remaining steady-state work
                if jj in (69, 73, 77):
                    fold_bank((jj - 69) // 4)
                if jj == 67:
                    # rowacc for jj<64 is final (service(65) ran): ship it
                    nc.sync.dma_start(rowacc_d[:, 0:128], rowacc[:, 0:128])
            while pending:
                service(pending.popleft())
            fold_bank(3)

            nc.sync.dma_start(aqpq_d[0:1, :], aqpq_s[0:1, :])
            nc.sync.dma_start(aqpq_d[1:2, :], aqpq_s[32:33, :])
            nc.sync.dma_start(rowacc_d[:, 128:], rowacc[:, 128:])
            nc.sync.dma_start(poskey_d[:], poskey[:])

    nc.compile()
    return nc, PW


def _compute_dmax(lab_s):
    first = lab_s.reshape(NBLK, 128)[:, 0]
    last = lab_s.reshape(NBLK, 128)[:, -1]
    dmax = 0
    for jj in range(NBLK):
        i = jj
        while i > 0 and last[i - 1] >= first[jj]:
            i -= 1
        dmax = max(dmax, jj - i)
    return max(1, min(dmax, 63))


def get_program(dmax):
    key = ("v5", dmax)
    if key not in _prog_cache:
        _prog_cache[key] = _build_program(dmax)
    return _prog_cache[key]


def make_in_maps(embeddings, partition_labels):
    emb = np.asarray(embeddings, dtype=np.float32)
    labels = np.asarray(partition_labels).astype(np.int64)
    perm = np.argsort(labels, kind="stable")
    E_s = emb[perm]
    lab_s = labels[perm]
    lab_f = lab_s.astype(np.float32)

    dmax = _compute_dmax(lab_s)
    E_sT = np.ascontiguousarray(E_s.T).astype(BF16)
    dia = np.exp(np.sum(E_s.astype(np.float64) ** 2, axis=1) / TEMP)

    cls = np.arange(64, dtype=np.int64)
    in_maps = []
    for c in range(NC):
        idx = (np.arange(N) + c * RPC) % N
        ktrot = np.ascontiguousarray(E_sT[:, idx])
        kl = lab_f[idx[:NJJ * 128]].reshape(NJJ, 128).T
        koh = (lab_s[idx[:NJJ * 128]].reshape(NJJ, 128)[:, :, None]
               == cls[None, None, :])
        koh = np.ascontiguousarray(
            koh.transpose(1, 0, 2).reshape(128, NJJ * 64)).astype(BF16)
        qlab_c = lab_f[c * RPC:(c + 1) * RPC]
        qlabb = np.ascontiguousarray(
            np.broadcast_to(qlab_c.astype(BF16)[None, :], (128, RPC)))
        qoh = (lab_s[c * RPC:(c + 1) * RPC][None, :] == cls[:, None])
        qoh2 = np.ascontiguousarray(
            np.vstack([qoh, qoh])).astype(BF16)      # [128, RPC]
        in_maps.append({
            "kt": ktrot,
            "koh": koh,
            "klab": np.ascontiguousarray(kl).astype(np.float32),
            "qlabb": qlabb,
            "qoh": qoh2,
        })
    return in_maps, lab_s, dmax, dia


def combine(results, lab_s, PW, dia):
    A = np.zeros(N, dtype=np.float64)
    P = np.zeros(N, dtype=np.float64)
    for c, r in enumerate(results):
        base = c * RPC
        aqpq = np.asarray(r["aqpq"], dtype=np.float64)
        A[base:base + RPC] += aqpq[0]
        P[base:base + RPC] += aqpq[1]
        ra2 = np.asarray(r["rowacc2"], dtype=np.float64)
        ra = ra2[:, 0::2] + ra2[:, 1::2]
        pk = np.asarray(r["poskey"], dtype=np.float64)
        for jj in range(NJJ - 1):
            g = (base + jj * 128) % N
            A[g:g + 128] += ra[:, jj]
            if jj < PW:
                P[g:g + 128] += pk[:, jj]
    A -= dia
    P -= dia

    counts = np.bincount(lab_s, minlength=1)
    valid = counts[lab_s] >= 2
    n_valid = int(valid.sum())
    if n_valid == 0:
        return np.float32(0.0)
    loss = np.log(A) - np.log(np.maximum(P, 1e-300))
    return np.float32(loss[valid].sum() / n_valid)


def kernel(embeddings, partition_labels):
    from concourse.bass_utils import run_bass_kernel_spmd

    in_maps, lab_s, dmax, dia = make_in_maps(embeddings, partition_labels)
    nc, PW = get_program(dmax)
    res = run_bass_kernel_spmd(nc, in_maps, list(range(NC)))
    return combine(res.results, lab_s, PW, dia)


# revision 27
# speedup vs baseline: 1.0457x; 1.0457x over previous
"""v5: v4 + head/tail overhaul from trace analysis.

Trace findings on v4 (189.4us):
- scalar (ACT) is the pacer: 100% busy 38us..157us; head has ~14us of
  ACT idle (koh DMA queued behind 2MB of kt tiles -> 8.4us full-pipe
  stall), tail has ~13us after the last exp (fp32 LOW_HIGH ones-fold
  matmuls ~8.5us + serialized drains/DMAs), plus ~16us fixed NEFF
  pre/postamble.

v5 changes:
- DMA order: kq0 split (first 128 cols land first), kq1, koh_a (first
  16 jj), kq2, kq3, klab, qlabb, kts1, koh_b, kts2-4, qoh (bf16).
  Everything lands >=1us before its first consumer.
- LAG taper: 8 for jj<12 shrinking to 2 by jj=18, so the first oh
  matmul enters the PE queue only after koh_a has landed; exp pool
  widened to 10 bufs.
- init_S segmented: one zoh matmul after do_jj(2,4,6,8) instead of a
  2us block that starved ACT at startup.
- rowacc2 [128, 2*(NJJ-1)]: DVE cache-reduce accum -> col 2jj, ACT
  activation accum -> col 2jj+1, memset once, host sums the pair.
  Removes 79 TENSOR_REDUCEs and the racc indirection.
- folds in bf16 inside the steady state: per drained bank, S bank ->
  bf16 sc_b, pm_b = sc_b * qoh (TT bf16 2x), two bf16 ones-matmuls
  (aq/pq) into S rows 0/1 of the dead bank, [2,512] copy to SBUF.
  Replaces the tail's fp32 folds entirely.
- output DMAs chunked/early: rowacc2 cols for jj<64 DMA'd mid-kernel.
"""

import sys

if "/opt/trn_rl_repo" not in sys.path:
    sys.path.insert(0, "/opt/trn_rl_repo")

from collections import deque

import numpy as np
import ml_dtypes

N = 16384
D = 128
NC = 8
RPC = N // NC
QB = RPC // 128
NBLK = N // 128
NJJ = 80
TEMP = 0.5
BF16 = ml_dtypes.bfloat16

# Schraudolph exp in bf16 bits: bits = round(x*(128/ln2) + 128*(127-c)).
# x = psum/TEMP, folded into the multiplier. Row-sum ratio error ~0.1%
# and the ~+0.3% mean bias cancels between pos_sum and all_sum.
SCH_A = (1.0 / TEMP) * 128.0 / float(np.log(2.0))
SCH_B = 128.0 * (127.0 - 0.0436)

_prog_cache = {}


def _seg512(a, b):
    """Split [a, b) at absolute multiples of 512 (matmul ISA max width)."""
    out = []
    while a < b:
        n = min((a // 512 + 1) * 512, b) - a
        out.append((a, a + n))
        a += n
    return out


def _build_program(dmax):
    import concourse.bacc as bacc
    import concourse.tile as tile
    import concourse.mybir as mybir

    dt = mybir.dt
    AF = mybir.ActivationFunctionType
    ALU = mybir.AluOpType

    PW = min(NJJ - 1, QB + dmax)
    KOH_A = 16 * 64  # one-hot cols for jj 0..15

    nc = bacc.Bacc(
        "TRN2",
        target_bir_lowering=False,
        debug=False,
        enable_asserts=False,
        num_devices=NC,
    )

    kt_d = nc.dram_tensor("kt", [D, N], dt.bfloat16, kind="ExternalInput").ap()
    koh_d = nc.dram_tensor("koh", [128, NJJ * 64], dt.bfloat16, kind="ExternalInput").ap()
    klab_d = nc.dram_tensor("klab", [128, NJJ], dt.float32, kind="ExternalInput").ap()
    qlabb_d = nc.dram_tensor("qlabb", [128, RPC], dt.bfloat16, kind="ExternalInput").ap()
    qoh_d = nc.dram_tensor("qoh", [128, RPC], dt.bfloat16, kind="ExternalInput").ap()

    rowacc_d = nc.dram_tensor(
        "rowacc2", [128, 2 * (NJJ - 1)], dt.float32, kind="ExternalOutput").ap()
    poskey_d = nc.dram_tensor("poskey", [128, PW], dt.float32, kind="ExternalOutput").ap()
    aqpq_d = nc.dram_tensor("aqpq", [2, RPC], dt.float32, kind="ExternalOutput").ap()

    with tile.TileContext(nc) as tc:
        with (
            tc.tile_pool(name="keys", bufs=1) as keys_pool,
            tc.tile_pool(name="aux", bufs=1) as aux_pool,
            tc.tile_pool(name="ps", bufs=2, space="PSUM") as psum_pool,
            tc.tile_pool(name="sacc", bufs=1, space="PSUM") as sacc_pool,
            tc.tile_pool(name="ex", bufs=10) as exp_pool,
            tc.tile_pool(name="jk", bufs=2) as junk_pool,
            tc.tile_pool(name="fin", bufs=1) as fin_pool,
        ):
            # --- input DMAs, deadline-ordered on the sync HWDGE ring ---
            kq = keys_pool.tile([D, 2048], dt.bfloat16, tag="kq", name="kq")
            koh_s = aux_pool.tile([128, NJJ * 64], dt.bfloat16, tag="koh")
            klab_s = aux_pool.tile([128, NJJ], dt.float32, tag="klab")
            qlabb_s = aux_pool.tile([128, RPC], dt.bfloat16, tag="qlabb")
            qoh_s = aux_pool.tile([128, RPC], dt.bfloat16, tag="qoh")
            kts = [None] + [
                keys_pool.tile([D, 2048], dt.bfloat16, tag=f"kt{j}", name=f"kt{j}")
                for j in range(1, 5)]

            nc.sync.dma_start(kq[:, 0:128], kt_d[:, 0:128])
            nc.sync.dma_start(kq[:, 128:512], kt_d[:, 128:512])
            nc.sync.dma_start(kq[:, 512:1024], kt_d[:, 512:1024])
            nc.sync.dma_start(koh_s[:, 0:KOH_A], koh_d[:, 0:KOH_A])
            nc.sync.dma_start(kq[:, 1024:1536], kt_d[:, 1024:1536])
            nc.sync.dma_start(kq[:, 1536:2048], kt_d[:, 1536:2048])
            nc.sync.dma_start(klab_s[:], klab_d[:])
            nc.sync.dma_start(qlabb_s[:], qlabb_d[:])
            nc.sync.dma_start(kts[1][:], kt_d[:, 2048:4096])
            nc.sync.dma_start(koh_s[:, KOH_A:], koh_d[:, KOH_A:])
            nc.sync.dma_start(kts[2][:], kt_d[:, 4096:6144])
            nc.sync.dma_start(kts[3][:], kt_d[:, 6144:8192])
            nc.sync.dma_start(kts[4][:], kt_d[:, 8192:10240])
            nc.sync.dma_start(qoh_s[:], qoh_d[:])

            def kt_block(b):  # [128, 128] lhsT slice for key block b
                col = b * 128
                if col < 2048:
                    return kq[:, col:col + 128]
                return kts[col // 2048][:, col % 2048:col % 2048 + 128]

            zoh = aux_pool.tile([128, 128], dt.bfloat16, tag="zoh")
            nc.vector.memset(zoh[:], 0.0)
            ones128 = aux_pool.tile([128, 1], dt.bfloat16, tag="ones128")
            nc.vector.memset(ones128[:], 1.0)

            rowacc = fin_pool.tile([128, 2 * (NJJ - 1)], dt.float32, tag="rowacc")
            nc.vector.memset(rowacc[:], 0.0)
            poskey = fin_pool.tile([128, PW], dt.float32, tag="poskey")

            # doubled S accumulator: rows 0-63 and 64-127 are independent
            # halves (col-group tiling); folded per-bank by ones-matmuls.
            S = sacc_pool.tile([128, RPC], dt.float32, tag="sacc")

            def init_S_seg(t):
                nc.tensor.matmul(
                    S[:, t * 512:(t + 1) * 512], zoh[:],
                    kq[:, 0:512],
                    start=True, stop=False, skip_group_check=True,
                )

            oh_ctr = [0]

            def oh_mm(jj, lo, a, b, ex, stop):
                for (s0, s1) in _seg512(a, b):
                    th = oh_ctr[0] % 2
                    oh_ctr[0] += 1
                    nc.tensor.matmul(
                        S[th * 64:(th + 1) * 64, s0:s1],
                        koh_s[:, jj * 64:(jj + 1) * 64],
                        ex[:, s0 - lo:s1 - lo],
                        start=False, stop=stop,
                        skip_group_check=True,
                        tile_position=(0, th * 64),
                    )

            pending = deque()

            def mode_for(jj):
                # 'b': exact, window-1 rowsum on DVE cache-reduce (early,
                #      poskey band jj; also keeps the diagonal exact)
                # 'A': window-2 exp on DVE (Schraudolph bits; window-1 stays
                #      on ACT with accum so the psum ring never waits on the
                #      DVE), window-2 rowsum via DVE cache-reduce
                # 'a': window-1 exp on DVE (for jj>=64 whose window-1 holds
                #      the extra region and can't use the ACT accum)
                # 'c': exact, BOTH windows' rowsums on ACT accumulators;
                #      frees ~1200ns of DVE per jj
                if jj < 20:
                    return 'b'
                if jj >= 72:
                    # single-window tail jj: offload would serialize on DVE,
                    # 'c' would pollute rowacc with the extra region
                    return 'b'
                if jj >= 64:
                    return 'a'
                return 'a' if jj % 3 != 2 else 'c'

            def service(rec):
                jj, lo, end, acc_lo, oh_end, stt_lo, ex, mode = rec
                # one-hot class sums: extra (d=64) part full weight, main
                # part excludes the d=0 block
                if lo < acc_lo:
                    oh_mm(jj, lo, lo, acc_lo, ex, stop=True)
                if acc_lo < oh_end:
                    oh_mm(jj, lo, acc_lo, oh_end, ex, stop=False)
                # per-key row sum, first window's share on the DVE (the
                # second window accumulated on ACT during exp)
                d1 = 1024 if (lo < 1024 and end > 1024) else end
                if mode == 'A':
                    # window-2 was Schraudolph'd on the DVE: its rowsum too
                    junk = junk_pool.tile([128, 2048], dt.bfloat16, tag="jk", name="jk")
                    nc.vector.tensor_scalar(
                        junk[:, :end - d1], ex[:, d1 - lo:end - lo],
                        1.0, 0.0, ALU.mult, ALU.add,
                        accum_out=rowacc[:, 2 * jj + 1:2 * jj + 2],
                    )
                elif mode != 'c' and acc_lo < d1:
                    junk = junk_pool.tile([128, 2048], dt.bfloat16, tag="jk", name="jk")
                    nc.vector.tensor_scalar(
                        junk[:, :d1 - acc_lo], ex[:, acc_lo - lo:d1 - lo],
                        1.0, 0.0, ALU.mult, ALU.add,
                        accum_out=rowacc[:, 2 * jj:2 * jj + 1],
                    )
                # per-key positive sum over the same-class band
                if stt_lo is not None and stt_lo < end:
                    mk = junk_pool.tile([128, 2048], dt.bfloat16, tag="jk", name="mk")
                    nc.vector.scalar_tensor_tensor(
                        mk[:, :end - stt_lo], qlabb_s[:, stt_lo:end],
                        klab_s[:, jj:jj + 1],
                        ex[:, stt_lo - lo:end - lo],
                        ALU.is_equal, ALU.mult,
                        accum_out=poskey[:, jj:jj + 1],
                    )

            def lag_for(jj):
                if jj >= 77:
                    # drain the pipeline early so the tail services overlap
                    # the last exps instead of running after them
                    return 1
                return max(2, min(8, 8 - (jj - 11)))

            def do_jj(jj):
                main_lo = max(0, jj - 63)
                hi = min(QB - 1, jj)
                lo = (jj - 64 if jj >= 64 else main_lo) * 128
                end = (hi + 1) * 128
                acc_lo = main_lo * 128
                oh_end = min(end, jj * 128) if jj <= QB - 1 else end
                stt_lo = max(acc_lo, (jj - dmax) * 128) if jj <= QB - 1 + dmax else None

                mode = mode_for(jj)
                ex = exp_pool.tile([128, 2048], dt.bfloat16, tag="ex", name="ex")
                k = 0
                w0 = (lo // 1024) * 1024
                while w0 < end:
                    p_lo = max(w0, lo)
                    p_end = min(w0 + 1024, end)
                    if p_lo < p_end:
                        ps = psum_pool.tile([128, 1024], dt.float32, tag="ps", name="ps")
                        for (s0, s1) in _seg512(p_lo, p_end):
                            nc.tensor.matmul(
                                ps[:, s0 - w0:s1 - w0],
                                kt_block(jj), kq[:, s0:s1],
                            )
                        if k == 1 and mode == 'A':
                            # Schraudolph window-2 on the DVE: window-1's
                            # psum consumer stays the (fast) ACT
                            nc.vector.tensor_scalar(
                                ex[:, p_lo - lo:p_end - lo].bitcast(dt.int16),
                                ps[:, p_lo - w0:p_end - w0],
                                SCH_A, SCH_B, ALU.mult, ALU.add,
                            )
                        elif k == 1 and p_lo >= acc_lo:
                            # second window: row-sum for free on the ACT accum
                            nc.scalar.activation(
                                ex[:, p_lo - lo:p_end - lo],
                                ps[:, p_lo - w0:p_end - w0],
                                AF.Exp, scale=1.0 / TEMP,
                                accum_out=rowacc[:, 2 * jj + 1:2 * jj + 2],
                            )
                        elif k == 0 and mode == 'A':
                            # first window: row-sum on the ACT accum
                            nc.scalar.activation(
                                ex[:, p_lo - lo:p_end - lo],
                                ps[:, p_lo - w0:p_end - w0],
                                AF.Exp, scale=1.0 / TEMP,
                                accum_out=rowacc[:, 2 * jj:2 * jj + 1],
                            )
                        elif k == 0 and mode == 'a':
                            # Schraudolph: bf16 bits via DVE fma + int16
                            # round, frees the ACT for other windows
                            nc.vector.tensor_scalar(
                                ex[:, p_lo - lo:p_end - lo].bitcast(dt.int16),
                                ps[:, p_lo - w0:p_end - w0],
                                SCH_A, SCH_B, ALU.mult, ALU.add,
                            )
                        elif k == 0 and mode == 'c':
                            # first window: row-sum on the ACT accum too
                            nc.scalar.activation(
                                ex[:, p_lo - lo:p_end - lo],
                                ps[:, p_lo - w0:p_end - w0],
                                AF.Exp, scale=1.0 / TEMP,
                                accum_out=rowacc[:, 2 * jj:2 * jj + 1],
                            )
                        else:
                            nc.scalar.activation(
                                ex[:, p_lo - lo:p_end - lo],
                                ps[:, p_lo - w0:p_end - w0],
                                AF.Exp, scale=1.0 / TEMP,
                            )
                        k += 1
                    w0 += 1024
                pending.append((jj, lo, end, acc_lo, oh_end, stt_lo, ex, mode))
                while len(pending) > lag_for(jj):
                    service(pending.popleft())

            aqpq_s = fin_pool.tile([33, RPC], dt.float32, tag="aqpqs")

            def fold_bank(b):
                sl = slice(b * 512, (b + 1) * 512)
                sc_b = fin_pool.tile([128, 512], dt.bfloat16, tag=f"sc{b}")
                nc.vector.tensor_copy(sc_b[:], S[:, sl])
                pm_b = fin_pool.tile([128, 512], dt.bfloat16, tag=f"pm{b}")
                nc.vector.tensor_mul(pm_b[:], sc_b[:], qoh_s[:, sl])
                # fold halves+classes via bf16 ones-matmuls into the dead
                # bank's rows 0 (all-sum) and 32 (pos-sum; matmul output
                # base partition must be 0/32/64)
                nc.tensor.matmul(
                    S[0:1, sl], ones128[:], sc_b[:],
                    start=True, stop=True, skip_group_check=True)
                nc.tensor.matmul(
                    S[32:33, sl], ones128[:], pm_b[:],
                    start=True, stop=True, skip_group_check=True)
                # one copy spanning rows 0..32 costs the same as [1,512]
                # (free-dim bound); rows 1..31 are don't-care
                nc.vector.tensor_copy(aqpq_s[:, sl], S[0:33, sl])

            do_jj(0)
            for jj in range(1, NJJ):
                do_jj(jj)
                if jj in (2, 4, 6, 8):
                    init_S_seg((jj - 2) // 2)
                # S bank b (query cols [b*512,(b+1)*512)) is final after
                # oh(67+4b), serviced during do_jj(69+4b): fold right away,
                # hidden under the remaining steady-state work
                if jj in (69, 73, 77):
                    fold_bank((jj - 69) // 4)
                if jj == 67:
                    # rowacc for jj<64 is final (service(65) ran): ship it
                    nc.sync.dma_start(rowacc_d[:, 0:128], rowacc[:, 0:128])
            while pending:
                service(pending.popleft())
            fold_bank(3)

            nc.sync.dma_start(aqpq_d[0:1, :], aqpq_s[0:1, :])
            nc.sync.dma_start(aqpq_d[1:2, :], aqpq_s[32:33, :])
            nc.sync.dma_start(rowacc_d[:, 128:], rowacc[:, 128:])
            nc.sync.dma_start(poskey_d[:], poskey[:])

    nc.compile()
    return nc, PW


def _compute_dmax(lab_s):
    first = lab_s.reshape(NBLK, 128)[:, 0]
    last = lab_s.reshape(NBLK, 128)[:, -1]
    dmax = 0
    for jj in range(NBLK):
        i = jj
        while i > 0 and last[i - 1] >= first[jj]:
            i -= 1
        dmax = max(dmax, jj - i)
    return max(1, min(dmax, 63))


def get_program(dmax):
    key = ("v5", dmax)
    if key not in _prog_cache:
        _prog_cache[key] = _build_program(dmax)
    return _prog_cache[key]


def make_in_maps(embeddings, partition_labels):
    emb = np.asarray(embeddings, dtype=np.float32)
    labels = np.asarray(partition_labels).astype(np.int64)
    perm = np.argsort(labels, kind="stable")
    E_s = emb[perm]
    lab_s = labels[perm]
    lab_f = lab_s.astype(np.float32)

    dmax = _compute_dmax(lab_s)
    E_sT = np.ascontiguousarray(E_s.T).astype(BF16)
    dia = np.exp(np.sum(E_s.astype(np.float64) ** 2, axis=1) / TEMP)

    cls = np.arange(64, dtype=np.int64)
    in_maps = []
    for c in range(NC):
        idx = (np.arange(N) + c * RPC) % N
        ktrot = np.ascontiguousarray(E_sT[:, idx])
        kl = lab_f[idx[:NJJ * 128]].reshape(NJJ, 128).T
        koh = (lab_s[idx[:NJJ * 128]].reshape(NJJ, 128)[:, :, None]
               == cls[None, None, :])
        koh = np.ascontiguousarray(
            koh.transpose(1, 0, 2).reshape(128, NJJ * 64)).astype(BF16)
        qlab_c = lab_f[c * RPC:(c + 1) * RPC]
        qlabb = np.ascontiguousarray(
            np.broadcast_to(qlab_c.astype(BF16)[None, :], (128, RPC)))
        qoh = (lab_s[c * RPC:(c + 1) * RPC][None, :] == cls[:, None])
        qoh2 = np.ascontiguousarray(
            np.vstack([qoh, qoh])).astype(BF16)      # [128, RPC]
        in_maps.append({
            "kt": ktrot,
            "koh": koh,
            "klab": np.ascontiguousarray(kl).astype(np.float32),
            "qlabb": qlabb,
            "qoh": qoh2,
        })
    return in_maps, lab_s, dmax, dia


def combine(results, lab_s, PW, dia):
    A = np.zeros(N, dtype=np.float64)
    P = np.zeros(N, dtype=np.float64)
    for c, r in enumerate(results):
        base = c * RPC
        aqpq = np.asarray(r["aqpq"], dtype=np.float64)
        A[base:base + RPC] += aqpq[0]
        P[base:base + RPC] += aqpq[1]
        ra2 = np.asarray(r["rowacc2"], dtype=np.float64)
        ra = ra2[:, 0::2] + ra2[:, 1::2]
        pk = np.asarray(r["poskey"], dtype=np.float64)
        for jj in range(NJJ - 1):
            g = (base + jj * 128) % N
            A[g:g + 128] += ra[:, jj]
            if jj < PW:
                P[g:g + 128] += pk[:, jj]
    A -= dia
    P -= dia

    counts = np.bincount(lab_s, minlength=1)
    valid = counts[lab_s] >= 2
    n_valid = int(valid.sum())
    if n_valid == 0:
        return np.float32(0.0)
    loss = np.log(A) - np.log(np.maximum(P, 1e-300))
    return np.float32(loss[valid].sum() / n_valid)


def kernel(embeddings, partition_labels):
    from concourse.bass_utils import run_bass_kernel_spmd

    in_maps, lab_s, dmax, dia = make_in_maps(embeddings, partition_labels)
    nc, PW = get_program(dmax)
    res = run_bass_kernel_spmd(nc, in_maps, list(range(NC)))
    return combine(res.results, lab_s, PW, dia)


# revision 34
# speedup vs baseline: 1.1201x; 1.0711x over previous
"""v5: v4 + head/tail overhaul from trace analysis.

Trace findings on v4 (189.4us):
- scalar (ACT) is the pacer: 100% busy 38us..157us; head has ~14us of
  ACT idle (koh DMA queued behind 2MB of kt tiles -> 8.4us full-pipe
  stall), tail has ~13us after the last exp (fp32 LOW_HIGH ones-fold
  matmuls ~8.5us + serialized drains/DMAs), plus ~16us fixed NEFF
  pre/postamble.

v5 changes:
- DMA order: kq0 split (first 128 cols land first), kq1, koh_a (first
  16 jj), kq2, kq3, klab, qlabb, kts1, koh_b, kts2-4, qoh (bf16).
  Everything lands >=1us before its first consumer.
- LAG taper: 8 for jj<12 shrinking to 2 by jj=18, so the first oh
  matmul enters the PE queue only after koh_a has landed; exp pool
  widened to 10 bufs.
- init_S segmented: one zoh matmul after do_jj(2,4,6,8) instead of a
  2us block that starved ACT at startup.
- rowacc2 [128, 2*(NJJ-1)]: DVE cache-reduce accum -> col 2jj, ACT
  activation accum -> col 2jj+1, memset once, host sums the pair.
  Removes 79 TENSOR_REDUCEs and the racc indirection.
- folds in bf16 inside the steady state: per drained bank, S bank ->
  bf16 sc_b, pm_b = sc_b * qoh (TT bf16 2x), two bf16 ones-matmuls
  (aq/pq) into S rows 0/1 of the dead bank, [2,512] copy to SBUF.
  Replaces the tail's fp32 folds entirely.
- output DMAs chunked/early: rowacc2 cols for jj<64 DMA'd mid-kernel.
"""

import sys

if "/opt/trn_rl_repo" not in sys.path:
    sys.path.insert(0, "/opt/trn_rl_repo")

from collections import deque

import numpy as np
import ml_dtypes

N = 16384
D = 128
NC = 8
RPC = N // NC
QB = RPC // 128
NBLK = N // 128
NJJ = 80
TEMP = 0.5
BF16 = ml_dtypes.bfloat16

# Schraudolph exp in bf16 bits: bits = round(x*(128/ln2) + 128*(127-c)).
# x = psum/TEMP, folded into the multiplier. Row-sum ratio error ~0.1%
# and the ~+0.3% mean bias cancels between pos_sum and all_sum.
SCH_A = (1.0 / TEMP) * 128.0 / float(np.log(2.0))
SCH_B = 128.0 * (127.0 - 0.0436)

_prog_cache = {}


def _seg512(a, b):
    """Split [a, b) at absolute multiples of 512 (matmul ISA max width)."""
    out = []
    while a < b:
        n = min((a // 512 + 1) * 512, b) - a
        out.append((a, a + n))
        a += n
    return out


def _build_program(dmax):
    import concourse.bacc as bacc
    import concourse.tile as tile
    import concourse.mybir as mybir

    dt = mybir.dt
    AF = mybir.ActivationFunctionType
    ALU = mybir.AluOpType

    PW = min(NJJ - 1, QB + dmax)
    KOH_A = 16 * 64  # one-hot cols for jj 0..15

    nc = bacc.Bacc(
        "TRN2",
        target_bir_lowering=False,
        debug=False,
        enable_asserts=False,
        num_devices=NC,
    )

    kt_d = nc.dram_tensor("kt", [D, N], dt.bfloat16, kind="ExternalInput").ap()
    koh_d = nc.dram_tensor("koh", [128, NJJ * 64], dt.bfloat16, kind="ExternalInput").ap()
    klab_d = nc.dram_tensor("klab", [128, NJJ], dt.float32, kind="ExternalInput").ap()
    qlabb_d = nc.dram_tensor("qlabb", [128, RPC], dt.bfloat16, kind="ExternalInput").ap()
    qoh_d = nc.dram_tensor("qoh", [128, RPC], dt.bfloat16, kind="ExternalInput").ap()

    rowacc_d = nc.dram_tensor(
        "rowacc2", [128, 2 * (NJJ - 1)], dt.float32, kind="ExternalOutput").ap()
    poskey_d = nc.dram_tensor("poskey", [128, PW], dt.float32, kind="ExternalOutput").ap()
    # rows: 0=aq_lo 1=aq_hi 2=pq_lo 3=pq_hi; col = b*512 + (c % 512) for
    # S-bank b, where lo = queries [b*1024, b*1024+512), hi = +512
    aqpq_d = nc.dram_tensor("aqpq", [4, RPC // 2], dt.float32, kind="ExternalOutput").ap()

    with tile.TileContext(nc) as tc:
        with (
            tc.tile_pool(name="keys", bufs=1) as keys_pool,
            tc.tile_pool(name="aux", bufs=1) as aux_pool,
            tc.tile_pool(name="ps", bufs=3, space="PSUM") as psum_pool,
            tc.tile_pool(name="sacc", bufs=1, space="PSUM") as sacc_pool,
            tc.tile_pool(name="ex", bufs=10) as exp_pool,
            tc.tile_pool(name="jk", bufs=2) as junk_pool,
            tc.tile_pool(name="fin", bufs=1) as fin_pool,
        ):
            # --- input DMAs, deadline-ordered on the sync HWDGE ring ---
            kq = keys_pool.tile([D, 2048], dt.bfloat16, tag="kq", name="kq")
            koh_s = aux_pool.tile([128, NJJ * 64], dt.bfloat16, tag="koh")
            klab_s = aux_pool.tile([128, NJJ], dt.float32, tag="klab")
            qlabb_s = aux_pool.tile([128, RPC], dt.bfloat16, tag="qlabb")
            qoh_s = aux_pool.tile([128, RPC], dt.bfloat16, tag="qoh")
            kts = [None] + [
                keys_pool.tile([D, 2048], dt.bfloat16, tag=f"kt{j}", name=f"kt{j}")
                for j in range(1, 5)]

            nc.sync.dma_start(kq[:, 0:128], kt_d[:, 0:128])
            nc.sync.dma_start(kq[:, 128:512], kt_d[:, 128:512])
            nc.sync.dma_start(kq[:, 512:1024], kt_d[:, 512:1024])
            nc.sync.dma_start(koh_s[:, 0:KOH_A], koh_d[:, 0:KOH_A])
            nc.sync.dma_start(kq[:, 1024:1536], kt_d[:, 1024:1536])
            nc.sync.dma_start(kq[:, 1536:2048], kt_d[:, 1536:2048])
            nc.sync.dma_start(klab_s[:], klab_d[:])
            nc.sync.dma_start(qlabb_s[:], qlabb_d[:])
            nc.sync.dma_start(kts[1][:], kt_d[:, 2048:4096])
            nc.sync.dma_start(koh_s[:, KOH_A:], koh_d[:, KOH_A:])
            nc.sync.dma_start(kts[2][:], kt_d[:, 4096:6144])
            nc.sync.dma_start(kts[3][:], kt_d[:, 6144:8192])
            nc.sync.dma_start(kts[4][:], kt_d[:, 8192:10240])
            nc.sync.dma_start(qoh_s[:], qoh_d[:])

            def kt_block(b):  # [128, 128] lhsT slice for key block b
                col = b * 128
                if col < 2048:
                    return kq[:, col:col + 128]
                return kts[col // 2048][:, col % 2048:col % 2048 + 128]

            zoh = aux_pool.tile([128, 128], dt.bfloat16, tag="zoh")
            nc.vector.memset(zoh[:], 0.0)
            ones128 = aux_pool.tile([128, 1], dt.bfloat16, tag="ones128")
            nc.vector.memset(ones128[:], 1.0)

            rowacc = fin_pool.tile([128, 2 * (NJJ - 1)], dt.float32, tag="rowacc")
            nc.vector.memset(rowacc[:], 0.0)
            poskey = fin_pool.tile([128, PW], dt.float32, tag="poskey")

            # S packed into 2 PSUM banks [128, 1024]: query col c lives at
            # (rows h*64..h*64+63, col (c//1024)*512 + c%512) with
            # h = (c//512)%2 — frees 2 banks so the exp psum ring gets 3
            # buffers and the PE no longer waits directly on the window's
            # (possibly slow DVE) consumer.
            S = sacc_pool.tile([128, RPC // 2], dt.float32, tag="sacc")

            def init_S_seg(t):
                nc.tensor.matmul(
                    S[:, t * 512:(t + 1) * 512], zoh[:],
                    kq[:, 0:512],
                    start=True, stop=False, skip_group_check=True,
                )

            def oh_mm(jj, lo, a, b, ex, stop):
                for (s0, s1) in _seg512(a, b):
                    th = (s0 // 512) % 2
                    c0 = (s0 // 1024) * 512 + (s0 % 512)
                    nc.tensor.matmul(
                        S[th * 64:(th + 1) * 64, c0:c0 + (s1 - s0)],
                        koh_s[:, jj * 64:(jj + 1) * 64],
                        ex[:, s0 - lo:s1 - lo],
                        start=False, stop=stop,
                        skip_group_check=True,
                        tile_position=(0, th * 64),
                    )

            pending = deque()

            def mode_for(jj):
                # 'b': exact, window-1 rowsum on DVE cache-reduce (early,
                #      poskey band jj; also keeps the diagonal exact)
                # 'A': window-2 exp on DVE (Schraudolph bits; window-1 stays
                #      on ACT with accum so the psum ring never waits on the
                #      DVE), window-2 rowsum via DVE cache-reduce
                # 'a': window-1 exp on DVE (for jj>=64 whose window-1 holds
                #      the extra region and can't use the ACT accum)
                # 'c': exact, BOTH windows' rowsums on ACT accumulators;
                #      frees ~1200ns of DVE per jj
                if jj < 20:
                    return 'b'
                if jj >= 72:
                    # single-window tail jj: offload would serialize on DVE,
                    # 'c' would pollute rowacc with the extra region
                    return 'b'
                if jj >= 64:
                    return 'a'
                return 'a' if jj % 3 != 2 else 'c'

            def service(rec):
                jj, lo, end, acc_lo, oh_end, stt_lo, ex, mode = rec
                # one-hot class sums: extra (d=64) part full weight, main
                # part excludes the d=0 block
                if lo < acc_lo:
                    oh_mm(jj, lo, lo, acc_lo, ex, stop=True)
                if acc_lo < oh_end:
                    oh_mm(jj, lo, acc_lo, oh_end, ex, stop=False)
                # per-key row sum, first window's share on the DVE (the
                # second window accumulated on ACT during exp)
                d1 = 1024 if (lo < 1024 and end > 1024) else end
                if mode == 'A':
                    # window-2 was Schraudolph'd on the DVE: its rowsum too
                    junk = junk_pool.tile([128, 2048], dt.bfloat16, tag="jk", name="jk")
                    nc.vector.tensor_scalar(
                        junk[:, :end - d1], ex[:, d1 - lo:end - lo],
                        1.0, 0.0, ALU.mult, ALU.add,
                        accum_out=rowacc[:, 2 * jj + 1:2 * jj + 2],
                    )
                elif mode != 'c' and acc_lo < d1:
                    junk = junk_pool.tile([128, 2048], dt.bfloat16, tag="jk", name="jk")
                    nc.vector.tensor_scalar(
                        junk[:, :d1 - acc_lo], ex[:, acc_lo - lo:d1 - lo],
                        1.0, 0.0, ALU.mult, ALU.add,
                        accum_out=rowacc[:, 2 * jj:2 * jj + 1],
                    )
                # per-key positive sum over the same-class band
                if stt_lo is not None and stt_lo < end:
                    mk = junk_pool.tile([128, 2048], dt.bfloat16, tag="jk", name="mk")
                    nc.vector.scalar_tensor_tensor(
                        mk[:, :end - stt_lo], qlabb_s[:, stt_lo:end],
                        klab_s[:, jj:jj + 1],
                        ex[:, stt_lo - lo:end - lo],
                        ALU.is_equal, ALU.mult,
                        accum_out=poskey[:, jj:jj + 1],
                    )

            def lag_for(jj):
                if jj >= 77:
                    # drain the pipeline early so the tail services overlap
                    # the last exps instead of running after them
                    return 1
                return max(2, min(8, 8 - (jj - 11)))

            def do_jj(jj):
                main_lo = max(0, jj - 63)
                hi = min(QB - 1, jj)
                lo = (jj - 64 if jj >= 64 else main_lo) * 128
                end = (hi + 1) * 128
                acc_lo = main_lo * 128
                oh_end = min(end, jj * 128) if jj <= QB - 1 else end
                stt_lo = max(acc_lo, (jj - dmax) * 128) if jj <= QB - 1 + dmax else None

                mode = mode_for(jj)
                ex = exp_pool.tile([128, 2048], dt.bfloat16, tag="ex", name="ex")
                k = 0
                w0 = (lo // 1024) * 1024
                while w0 < end:
                    p_lo = max(w0, lo)
                    p_end = min(w0 + 1024, end)
                    if p_lo < p_end:
                        ps = psum_pool.tile([128, 1024], dt.float32, tag="ps", name="ps")
                        for (s0, s1) in _seg512(p_lo, p_end):
                            nc.tensor.matmul(
                                ps[:, s0 - w0:s1 - w0],
                                kt_block(jj), kq[:, s0:s1],
                            )
                        if k == 1 and mode == 'A':
                            # Schraudolph window-2 on the DVE: window-1's
                            # psum consumer stays the (fast) ACT
                            nc.vector.tensor_scalar(
                                ex[:, p_lo - lo:p_end - lo].bitcast(dt.int16),
                                ps[:, p_lo - w0:p_end - w0],
                                SCH_A, SCH_B, ALU.mult, ALU.add,
                            )
                        elif k == 1 and p_lo >= acc_lo:
                            # second window: row-sum for free on the ACT accum
                            nc.scalar.activation(
                                ex[:, p_lo - lo:p_end - lo],
                                ps[:, p_lo - w0:p_end - w0],
                                AF.Exp, scale=1.0 / TEMP,
                                accum_out=rowacc[:, 2 * jj + 1:2 * jj + 2],
                            )
                        elif k == 0 and mode == 'A':
                            # first window: row-sum on the ACT accum
                            nc.scalar.activation(
                                ex[:, p_lo - lo:p_end - lo],
                                ps[:, p_lo - w0:p_end - w0],
                                AF.Exp, scale=1.0 / TEMP,
                                accum_out=rowacc[:, 2 * jj:2 * jj + 1],
                            )
                        elif k == 0 and mode == 'a':
                            # Schraudolph: bf16 bits via DVE fma + int16
                            # round, frees the ACT for other windows
                            nc.vector.tensor_scalar(
                                ex[:, p_lo - lo:p_end - lo].bitcast(dt.int16),
                                ps[:, p_lo - w0:p_end - w0],
                                SCH_A, SCH_B, ALU.mult, ALU.add,
                            )
                        elif k == 0 and mode == 'c':
                            # first window: row-sum on the ACT accum too
                            nc.scalar.activation(
                                ex[:, p_lo - lo:p_end - lo],
                                ps[:, p_lo - w0:p_end - w0],
                                AF.Exp, scale=1.0 / TEMP,
                                accum_out=rowacc[:, 2 * jj:2 * jj + 1],
                            )
                        else:
                            nc.scalar.activation(
                                ex[:, p_lo - lo:p_end - lo],
                                ps[:, p_lo - w0:p_end - w0],
                                AF.Exp, scale=1.0 / TEMP,
                            )
                        k += 1
                    w0 += 1024
                pending.append((jj, lo, end, acc_lo, oh_end, stt_lo, ex, mode))
                while len(pending) > lag_for(jj):
                    service(pending.popleft())

            # per S-bank b (query cols [b*1024,(b+1)*1024)): the low half
            # (rows 0-63) holds q[b*1024 : b*1024+512] class sums, the high
            # half q[b*1024+512 : (b+1)*1024). Fold each half separately
            # with a ones64-matmul; outputs staged in the dead bank's rows
            # 0/32/64 in two waves (matmul out base must be 0/32/64).
            aqpq_s = fin_pool.tile([65, RPC // 2], dt.float32, tag="aqpqs")
            aqpq2_s = fin_pool.tile([1, RPC // 2], dt.float32, tag="aqpq2s")

            def fold_bank(b):
                sl = slice(b * 512, (b + 1) * 512)
                sc_b = fin_pool.tile([128, 512], dt.bfloat16, tag=f"sc{b}")
                nc.vector.tensor_copy(sc_b[:], S[:, sl])
                # qoh in S layout: rows 0-127 already stacked [qoh; qoh],
                # slice the matching query cols for each half
                pm_b = fin_pool.tile([128, 512], dt.bfloat16, tag=f"pm{b}")
                nc.vector.tensor_mul(
                    pm_b[0:64, :], sc_b[0:64, :],
                    qoh_s[0:64, b * 1024:b * 1024 + 512])
                nc.vector.tensor_mul(
                    pm_b[64:128, :], sc_b[64:128, :],
                    qoh_s[64:128, b * 1024 + 512:(b + 1) * 1024])
                # wave 1: aq_lo -> row 0, aq_hi -> row 32, pq_lo -> row 64
                nc.tensor.matmul(
                    S[0:1, sl], ones128[0:64], sc_b[0:64, :],
                    start=True, stop=True, skip_group_check=True)
                nc.tensor.matmul(
                    S[32:33, sl], ones128[64:128], sc_b[64:128, :],
                    start=True, stop=True, skip_group_check=True)
                nc.tensor.matmul(
                    S[64:65, sl], ones128[0:64], pm_b[0:64, :],
                    start=True, stop=True, skip_group_check=True)
                nc.vector.tensor_copy(aqpq_s[:, sl], S[0:65, sl])
                # wave 2: pq_hi -> row 0 (after the copy drained it)
                nc.tensor.matmul(
                    S[0:1, sl], ones128[64:128], pm_b[64:128, :],
                    start=True, stop=True, skip_group_check=True)
                nc.vector.tensor_copy(aqpq2_s[:, sl], S[0:1, sl])

            do_jj(0)
            for jj in range(1, NJJ):
                do_jj(jj)
                if jj in (2, 4):
                    init_S_seg((jj - 2) // 2)
                # S bank 0 (query cols [0,1024)) is final after oh(71),
                # serviced during do_jj(73): fold right away, hidden under
                # the remaining steady-state work
                if jj == 73:
                    fold_bank(0)
                if jj == 67:
                    # rowacc for jj<64 is final (service(65) ran): ship it
                    nc.sync.dma_start(rowacc_d[:, 0:128], rowacc[:, 0:128])
            while pending:
                service(pending.popleft())
            fold_bank(1)

            nc.sync.dma_start(aqpq_d[0:1, :], aqpq_s[0:1, :])
            nc.sync.dma_start(aqpq_d[1:2, :], aqpq_s[32:33, :])
            nc.sync.dma_start(aqpq_d[2:3, :], aqpq_s[64:65, :])
            nc.sync.dma_start(aqpq_d[3:4, :], aqpq2_s[0:1, :])
            nc.sync.dma_start(rowacc_d[:, 128:], rowacc[:, 128:])
            nc.sync.dma_start(poskey_d[:], poskey[:])

    nc.compile()
    return nc, PW


def _compute_dmax(lab_s):
    first = lab_s.reshape(NBLK, 128)[:, 0]
    last = lab_s.reshape(NBLK, 128)[:, -1]
    dmax = 0
    for jj in range(NBLK):
        i = jj
        while i > 0 and last[i - 1] >= first[jj]:
            i -= 1
        dmax = max(dmax, jj - i)
    return max(1, min(dmax, 63))


def get_program(dmax):
    key = ("v5", dmax)
    if key not in _prog_cache:
        _prog_cache[key] = _build_program(dmax)
    return _prog_cache[key]


def make_in_maps(embeddings, partition_labels):
    emb = np.asarray(embeddings, dtype=np.float32)
    labels = np.asarray(partition_labels).astype(np.int64)
    perm = np.argsort(labels, kind="stable")
    E_s = emb[perm]
    lab_s = labels[perm]
    lab_f = lab_s.astype(np.float32)

    dmax = _compute_dmax(lab_s)
    E_sT = np.ascontiguousarray(E_s.T).astype(BF16)
    dia = np.exp(np.sum(E_s.astype(np.float64) ** 2, axis=1) / TEMP)

    cls = np.arange(64, dtype=np.int64)
    in_maps = []
    for c in range(NC):
        idx = (np.arange(N) + c * RPC) % N
        ktrot = np.ascontiguousarray(E_sT[:, idx])
        kl = lab_f[idx[:NJJ * 128]].reshape(NJJ, 128).T
        koh = (lab_s[idx[:NJJ * 128]].reshape(NJJ, 128)[:, :, None]
               == cls[None, None, :])
        koh = np.ascontiguousarray(
            koh.transpose(1, 0, 2).reshape(128, NJJ * 64)).astype(BF16)
        qlab_c = lab_f[c * RPC:(c + 1) * RPC]
        qlabb = np.ascontiguousarray(
            np.broadcast_to(qlab_c.astype(BF16)[None, :], (128, RPC)))
        qoh = (lab_s[c * RPC:(c + 1) * RPC][None, :] == cls[:, None])
        qoh2 = np.ascontiguousarray(
            np.vstack([qoh, qoh])).astype(BF16)      # [128, RPC]
        in_maps.append({
            "kt": ktrot,
            "koh": koh,
            "klab": np.ascontiguousarray(kl).astype(np.float32),
            "qlabb": qlabb,
            "qoh": qoh2,
        })
    return in_maps, lab_s, dmax, dia


def combine(results, lab_s, PW, dia):
    A = np.zeros(N, dtype=np.float64)
    P = np.zeros(N, dtype=np.float64)
    for c, r in enumerate(results):
        base = c * RPC
        aqpq = np.asarray(r["aqpq"], dtype=np.float64)
        for b in range(2):
            sl = slice(b * 512, (b + 1) * 512)
            A[base + b * 1024:base + b * 1024 + 512] += aqpq[0, sl]
            A[base + b * 1024 + 512:base + (b + 1) * 1024] += aqpq[1, sl]
            P[base + b * 1024:base + b * 1024 + 512] += aqpq[2, sl]
            P[base + b * 1024 + 512:base + (b + 1) * 1024] += aqpq[3, sl]
        ra2 = np.asarray(r["rowacc2"], dtype=np.float64)
        ra = ra2[:, 0::2] + ra2[:, 1::2]
        pk = np.asarray(r["poskey"], dtype=np.float64)
        for jj in range(NJJ - 1):
            g = (base + jj * 128) % N
            A[g:g + 128] += ra[:, jj]
            if jj < PW:
                P[g:g + 128] += pk[:, jj]
    A -= dia
    P -= dia

    counts = np.bincount(lab_s, minlength=1)
    valid = counts[lab_s] >= 2
    n_valid = int(valid.sum())
    if n_valid == 0:
        return np.float32(0.0)
    loss = np.log(A) - np.log(np.maximum(P, 1e-300))
    return np.float32(loss[valid].sum() / n_valid)


def kernel(embeddings, partition_labels):
    from concourse.bass_utils import run_bass_kernel_spmd

    in_maps, lab_s, dmax, dia = make_in_maps(embeddings, partition_labels)
    nc, PW = get_program(dmax)
    res = run_bass_kernel_spmd(nc, in_maps, list(range(NC)))
    return combine(res.results, lab_s, PW, dia)


# revision 36
# speedup vs baseline: 1.1743x; 1.0484x over previous
"""v5: v4 + head/tail overhaul from trace analysis.

Trace findings on v4 (189.4us):
- scalar (ACT) is the pacer: 100% busy 38us..157us; head has ~14us of
  ACT idle (koh DMA queued behind 2MB of kt tiles -> 8.4us full-pipe
  stall), tail has ~13us after the last exp (fp32 LOW_HIGH ones-fold
  matmuls ~8.5us + serialized drains/DMAs), plus ~16us fixed NEFF
  pre/postamble.

v5 changes:
- DMA order: kq0 split (first 128 cols land first), kq1, koh_a (first
  16 jj), kq2, kq3, klab, qlabb, kts1, koh_b, kts2-4, qoh (bf16).
  Everything lands >=1us before its first consumer.
- LAG taper: 8 for jj<12 shrinking to 2 by jj=18, so the first oh
  matmul enters the PE queue only after koh_a has landed; exp pool
  widened to 10 bufs.
- init_S segmented: one zoh matmul after do_jj(2,4,6,8) instead of a
  2us block that starved ACT at startup.
- rowacc2 [128, 2*(NJJ-1)]: DVE cache-reduce accum -> col 2jj, ACT
  activation accum -> col 2jj+1, memset once, host sums the pair.
  Removes 79 TENSOR_REDUCEs and the racc indirection.
- folds in bf16 inside the steady state: per drained bank, S bank ->
  bf16 sc_b, pm_b = sc_b * qoh (TT bf16 2x), two bf16 ones-matmuls
  (aq/pq) into S rows 0/1 of the dead bank, [2,512] copy to SBUF.
  Replaces the tail's fp32 folds entirely.
- output DMAs chunked/early: rowacc2 cols for jj<64 DMA'd mid-kernel.
"""

import sys

if "/opt/trn_rl_repo" not in sys.path:
    sys.path.insert(0, "/opt/trn_rl_repo")

from collections import deque

import numpy as np
import ml_dtypes

N = 16384
D = 128
NC = 8
RPC = N // NC
QB = RPC // 128
NBLK = N // 128
NJJ = 80
TEMP = 0.5
BF16 = ml_dtypes.bfloat16

# Schraudolph exp in bf16 bits: bits = round(x*(128/ln2) + 128*(127-c)).
# x = psum/TEMP, folded into the multiplier. Row-sum ratio error ~0.1%
# and the ~+0.3% mean bias cancels between pos_sum and all_sum.
SCH_A = (1.0 / TEMP) * 128.0 / float(np.log(2.0))
SCH_B = 128.0 * (127.0 - 0.0436)

_prog_cache = {}


def _seg512(a, b):
    """Split [a, b) at absolute multiples of 512 (matmul ISA max width)."""
    out = []
    while a < b:
        n = min((a // 512 + 1) * 512, b) - a
        out.append((a, a + n))
        a += n
    return out


def _build_program(dmax):
    import concourse.bacc as bacc
    import concourse.tile as tile
    import concourse.mybir as mybir

    dt = mybir.dt
    AF = mybir.ActivationFunctionType
    ALU = mybir.AluOpType

    PW = min(NJJ - 1, QB + dmax)
    KOH_A = 16 * 64  # one-hot cols for jj 0..15

    nc = bacc.Bacc(
        "TRN2",
        target_bir_lowering=False,
        debug=False,
        enable_asserts=False,
        num_devices=NC,
    )

    kt_d = nc.dram_tensor("kt", [D, N], dt.bfloat16, kind="ExternalInput").ap()
    koh_d = nc.dram_tensor("koh", [128, NJJ * 64], dt.bfloat16, kind="ExternalInput").ap()
    klab_d = nc.dram_tensor("klab", [128, NJJ], dt.float32, kind="ExternalInput").ap()
    qlabb_d = nc.dram_tensor("qlabb", [128, RPC], dt.bfloat16, kind="ExternalInput").ap()
    qoh_d = nc.dram_tensor("qoh", [128, RPC], dt.bfloat16, kind="ExternalInput").ap()

    rowacc_d = nc.dram_tensor(
        "rowacc2", [128, 2 * (NJJ - 1)], dt.float32, kind="ExternalOutput").ap()
    poskey_d = nc.dram_tensor("poskey", [128, PW], dt.float32, kind="ExternalOutput").ap()
    # rows: 0=aq_lo 1=aq_hi 2=pq_lo 3=pq_hi; col = b*512 + (c % 512) for
    # S-bank b, where lo = queries [b*1024, b*1024+512), hi = +512
    aqpq_d = nc.dram_tensor("aqpq", [4, RPC // 2], dt.float32, kind="ExternalOutput").ap()

    with tile.TileContext(nc) as tc:
        with (
            tc.tile_pool(name="keys", bufs=1) as keys_pool,
            tc.tile_pool(name="aux", bufs=1) as aux_pool,
            tc.tile_pool(name="ps", bufs=3, space="PSUM") as psum_pool,
            tc.tile_pool(name="sacc", bufs=1, space="PSUM") as sacc_pool,
            tc.tile_pool(name="ex", bufs=10) as exp_pool,
            tc.tile_pool(name="jk", bufs=2) as junk_pool,
            tc.tile_pool(name="fin", bufs=1) as fin_pool,
        ):
            # --- input DMAs, deadline-ordered on the sync HWDGE ring ---
            kq = keys_pool.tile([D, 2048], dt.bfloat16, tag="kq", name="kq")
            koh_s = aux_pool.tile([128, NJJ * 64], dt.bfloat16, tag="koh")
            klab_s = aux_pool.tile([128, NJJ], dt.float32, tag="klab")
            qlabb_s = aux_pool.tile([128, RPC], dt.bfloat16, tag="qlabb")
            qoh_s = aux_pool.tile([128, RPC], dt.bfloat16, tag="qoh")
            kts = [None] + [
                keys_pool.tile([D, 2048], dt.bfloat16, tag=f"kt{j}", name=f"kt{j}")
                for j in range(1, 5)]

            nc.sync.dma_start(kq[:, 0:128], kt_d[:, 0:128])
            nc.sync.dma_start(kq[:, 128:512], kt_d[:, 128:512])
            nc.sync.dma_start(kq[:, 512:1024], kt_d[:, 512:1024])
            nc.sync.dma_start(koh_s[:, 0:KOH_A], koh_d[:, 0:KOH_A])
            nc.sync.dma_start(kq[:, 1024:1536], kt_d[:, 1024:1536])
            nc.sync.dma_start(kq[:, 1536:2048], kt_d[:, 1536:2048])
            nc.sync.dma_start(klab_s[:], klab_d[:])
            nc.sync.dma_start(qlabb_s[:], qlabb_d[:])
            nc.sync.dma_start(kts[1][:], kt_d[:, 2048:4096])
            nc.sync.dma_start(koh_s[:, KOH_A:], koh_d[:, KOH_A:])
            nc.sync.dma_start(kts[2][:], kt_d[:, 4096:6144])
            nc.sync.dma_start(kts[3][:], kt_d[:, 6144:8192])
            nc.sync.dma_start(kts[4][:], kt_d[:, 8192:10240])
            nc.sync.dma_start(qoh_s[:], qoh_d[:])

            def kt_block(b):  # [128, 128] lhsT slice for key block b
                col = b * 128
                if col < 2048:
                    return kq[:, col:col + 128]
                return kts[col // 2048][:, col % 2048:col % 2048 + 128]

            zoh = aux_pool.tile([128, 128], dt.bfloat16, tag="zoh")
            nc.vector.memset(zoh[:], 0.0)
            ones128 = aux_pool.tile([128, 1], dt.bfloat16, tag="ones128")
            nc.vector.memset(ones128[:], 1.0)

            rowacc = fin_pool.tile([128, 2 * (NJJ - 1)], dt.float32, tag="rowacc")
            nc.vector.memset(rowacc[:], 0.0)
            poskey = fin_pool.tile([128, PW], dt.float32, tag="poskey")

            # S packed into 2 PSUM banks [128, 1024]: query col c lives at
            # (rows h*64..h*64+63, col (c//1024)*512 + c%512) with
            # h = (c//512)%2 — frees 2 banks so the exp psum ring gets 3
            # buffers and the PE no longer waits directly on the window's
            # (possibly slow DVE) consumer.
            S = sacc_pool.tile([128, RPC // 2], dt.float32, tag="sacc")

            def init_S_seg(t):
                nc.tensor.matmul(
                    S[:, t * 512:(t + 1) * 512], zoh[:],
                    kq[:, 0:512],
                    start=True, stop=False, skip_group_check=True,
                )

            def oh_mm(jj, lo, a, b, ex, stop):
                for (s0, s1) in _seg512(a, b):
                    th = (s0 // 512) % 2
                    c0 = (s0 // 1024) * 512 + (s0 % 512)
                    nc.tensor.matmul(
                        S[th * 64:(th + 1) * 64, c0:c0 + (s1 - s0)],
                        koh_s[:, jj * 64:(jj + 1) * 64],
                        ex[:, s0 - lo:s1 - lo],
                        start=False, stop=stop,
                        skip_group_check=True,
                        tile_position=(0, th * 64),
                    )

            pending = deque()

            def mode_for(jj):
                # 'b': exact, window-1 rowsum on DVE cache-reduce (early,
                #      poskey band jj; also keeps the diagonal exact)
                # 'A': window-2 exp on DVE (Schraudolph bits; window-1 stays
                #      on ACT with accum so the psum ring never waits on the
                #      DVE), window-2 rowsum via DVE cache-reduce
                # 'a': window-1 exp on DVE (for jj>=64 whose window-1 holds
                #      the extra region and can't use the ACT accum)
                # 'c': exact, BOTH windows' rowsums on ACT accumulators;
                #      frees ~1200ns of DVE per jj
                if jj < 16:
                    return 'b'
                if jj < 20:
                    # both rowsums on ACT: keeps the early DVE backlog
                    # (stt + catch-up cache-reduces) off the critical path
                    return 'c'
                if jj >= 72:
                    # single-window tail jj: offload would serialize on DVE,
                    # 'c' would pollute rowacc with the extra region
                    return 'b'
                if jj >= 64:
                    return 'a'
                return 'a' if jj % 3 != 2 else 'c'

            def service(rec):
                jj, lo, end, acc_lo, oh_end, stt_lo, ex, mode = rec
                # one-hot class sums: extra (d=64) part full weight, main
                # part excludes the d=0 block
                if lo < acc_lo:
                    oh_mm(jj, lo, lo, acc_lo, ex, stop=True)
                if acc_lo < oh_end:
                    oh_mm(jj, lo, acc_lo, oh_end, ex, stop=False)
                # per-key row sum, first window's share on the DVE (the
                # second window accumulated on ACT during exp)
                d1 = 1024 if (lo < 1024 and end > 1024) else end
                if mode == 'A':
                    # window-2 was Schraudolph'd on the DVE: its rowsum too
                    junk = junk_pool.tile([128, 2048], dt.bfloat16, tag="jk", name="jk")
                    nc.vector.tensor_scalar(
                        junk[:, :end - d1], ex[:, d1 - lo:end - lo],
                        1.0, 0.0, ALU.mult, ALU.add,
                        accum_out=rowacc[:, 2 * jj + 1:2 * jj + 2],
                    )
                elif mode != 'c' and acc_lo < d1:
                    junk = junk_pool.tile([128, 2048], dt.bfloat16, tag="jk", name="jk")
                    nc.vector.tensor_scalar(
                        junk[:, :d1 - acc_lo], ex[:, acc_lo - lo:d1 - lo],
                        1.0, 0.0, ALU.mult, ALU.add,
                        accum_out=rowacc[:, 2 * jj:2 * jj + 1],
                    )
                # per-key positive sum over the same-class band
                if stt_lo is not None and stt_lo < end:
                    mk = junk_pool.tile([128, 2048], dt.bfloat16, tag="jk", name="mk")
                    nc.vector.scalar_tensor_tensor(
                        mk[:, :end - stt_lo], qlabb_s[:, stt_lo:end],
                        klab_s[:, jj:jj + 1],
                        ex[:, stt_lo - lo:end - lo],
                        ALU.is_equal, ALU.mult,
                        accum_out=poskey[:, jj:jj + 1],
                    )

            def lag_for(jj):
                if jj >= 77:
                    # drain the pipeline early so the tail services overlap
                    # the last exps instead of running after them
                    return 1
                return max(2, min(8, 8 - (jj - 11) // 2))

            def do_jj(jj):
                main_lo = max(0, jj - 63)
                hi = min(QB - 1, jj)
                lo = (jj - 64 if jj >= 64 else main_lo) * 128
                end = (hi + 1) * 128
                acc_lo = main_lo * 128
                oh_end = min(end, jj * 128) if jj <= QB - 1 else end
                stt_lo = max(acc_lo, (jj - dmax) * 128) if jj <= QB - 1 + dmax else None

                mode = mode_for(jj)
                ex = exp_pool.tile([128, 2048], dt.bfloat16, tag="ex", name="ex")
                k = 0
                w0 = (lo // 1024) * 1024
                while w0 < end:
                    p_lo = max(w0, lo)
                    p_end = min(w0 + 1024, end)
                    if p_lo < p_end:
                        ps = psum_pool.tile([128, 1024], dt.float32, tag="ps", name="ps")
                        for (s0, s1) in _seg512(p_lo, p_end):
                            nc.tensor.matmul(
                                ps[:, s0 - w0:s1 - w0],
                                kt_block(jj), kq[:, s0:s1],
                            )
                        if k == 1 and mode == 'A':
                            # Schraudolph window-2 on the DVE: window-1's
                            # psum consumer stays the (fast) ACT
                            nc.vector.tensor_scalar(
                                ex[:, p_lo - lo:p_end - lo].bitcast(dt.int16),
                                ps[:, p_lo - w0:p_end - w0],
                                SCH_A, SCH_B, ALU.mult, ALU.add,
                            )
                        elif k == 1 and p_lo >= acc_lo:
                            # second window: row-sum for free on the ACT accum
                            nc.scalar.activation(
                                ex[:, p_lo - lo:p_end - lo],
                                ps[:, p_lo - w0:p_end - w0],
                                AF.Exp, scale=1.0 / TEMP,
                                accum_out=rowacc[:, 2 * jj + 1:2 * jj + 2],
                            )
                        elif k == 0 and mode == 'A':
                            # first window: row-sum on the ACT accum
                            nc.scalar.activation(
                                ex[:, p_lo - lo:p_end - lo],
                                ps[:, p_lo - w0:p_end - w0],
                                AF.Exp, scale=1.0 / TEMP,
                                accum_out=rowacc[:, 2 * jj:2 * jj + 1],
                            )
                        elif k == 0 and mode == 'a':
                            # Schraudolph: bf16 bits via DVE fma + int16
                            # round, frees the ACT for other windows
                            nc.vector.tensor_scalar(
                                ex[:, p_lo - lo:p_end - lo].bitcast(dt.int16),
                                ps[:, p_lo - w0:p_end - w0],
                                SCH_A, SCH_B, ALU.mult, ALU.add,
                            )
                        elif k == 0 and mode == 'c':
                            # first window: row-sum on the ACT accum too
                            nc.scalar.activation(
                                ex[:, p_lo - lo:p_end - lo],
                                ps[:, p_lo - w0:p_end - w0],
                                AF.Exp, scale=1.0 / TEMP,
                                accum_out=rowacc[:, 2 * jj:2 * jj + 1],
                            )
                        else:
                            nc.scalar.activation(
                                ex[:, p_lo - lo:p_end - lo],
                                ps[:, p_lo - w0:p_end - w0],
                                AF.Exp, scale=1.0 / TEMP,
                            )
                        k += 1
                    w0 += 1024
                pending.append((jj, lo, end, acc_lo, oh_end, stt_lo, ex, mode))
                while len(pending) > lag_for(jj):
                    service(pending.popleft())

            # per S-bank b (query cols [b*1024,(b+1)*1024)): the low half
            # (rows 0-63) holds q[b*1024 : b*1024+512] class sums, the high
            # half q[b*1024+512 : (b+1)*1024). Fold each half separately
            # with a ones64-matmul; outputs staged in the dead bank's rows
            # 0/32/64 in two waves (matmul out base must be 0/32/64).
            aqpq_s = fin_pool.tile([65, RPC // 2], dt.float32, tag="aqpqs")
            aqpq2_s = fin_pool.tile([1, RPC // 2], dt.float32, tag="aqpq2s")

            def fold_bank(b):
                sl = slice(b * 512, (b + 1) * 512)
                sc_b = fin_pool.tile([128, 512], dt.bfloat16, tag=f"sc{b}")
                nc.vector.tensor_copy(sc_b[:], S[:, sl])
                # qoh in S layout: rows 0-127 already stacked [qoh; qoh],
                # slice the matching query cols for each half
                pm_b = fin_pool.tile([128, 512], dt.bfloat16, tag=f"pm{b}")
                nc.vector.tensor_mul(
                    pm_b[0:64, :], sc_b[0:64, :],
                    qoh_s[0:64, b * 1024:b * 1024 + 512])
                nc.vector.tensor_mul(
                    pm_b[64:128, :], sc_b[64:128, :],
                    qoh_s[64:128, b * 1024 + 512:(b + 1) * 1024])
                # wave 1: aq_lo -> row 0, aq_hi -> row 32, pq_lo -> row 64
                nc.tensor.matmul(
                    S[0:1, sl], ones128[0:64], sc_b[0:64, :],
                    start=True, stop=True, skip_group_check=True)
                nc.tensor.matmul(
                    S[32:33, sl], ones128[64:128], sc_b[64:128, :],
                    start=True, stop=True, skip_group_check=True)
                nc.tensor.matmul(
                    S[64:65, sl], ones128[0:64], pm_b[0:64, :],
                    start=True, stop=True, skip_group_check=True)
                nc.vector.tensor_copy(aqpq_s[:, sl], S[0:65, sl])
                # wave 2: pq_hi -> row 0 (after the copy drained it)
                nc.tensor.matmul(
                    S[0:1, sl], ones128[64:128], pm_b[64:128, :],
                    start=True, stop=True, skip_group_check=True)
                nc.vector.tensor_copy(aqpq2_s[:, sl], S[0:1, sl])

            do_jj(0)
            for jj in range(1, NJJ):
                do_jj(jj)
                if jj in (2, 4):
                    init_S_seg((jj - 2) // 2)
                # S bank 0 (query cols [0,1024)) is final after oh(71),
                # serviced during do_jj(73): fold right away, hidden under
                # the remaining steady-state work
                if jj == 73:
                    fold_bank(0)
                if jj == 67:
                    # rowacc for jj<64 is final (service(65) ran): ship it
                    nc.sync.dma_start(rowacc_d[:, 0:128], rowacc[:, 0:128])
            while pending:
                service(pending.popleft())
            fold_bank(1)

            nc.sync.dma_start(aqpq_d[0:1, :], aqpq_s[0:1, :])
            nc.sync.dma_start(aqpq_d[1:2, :], aqpq_s[32:33, :])
            nc.sync.dma_start(aqpq_d[2:3, :], aqpq_s[64:65, :])
            nc.sync.dma_start(aqpq_d[3:4, :], aqpq2_s[0:1, :])
            nc.sync.dma_start(rowacc_d[:, 128:], rowacc[:, 128:])
            nc.sync.dma_start(poskey_d[:], poskey[:])

    nc.compile()
    return nc, PW


def _compute_dmax(lab_s):
    first = lab_s.reshape(NBLK, 128)[:, 0]
    last = lab_s.reshape(NBLK, 128)[:, -1]
    dmax = 0
    for jj in range(NBLK):
        i = jj
        while i > 0 and last[i - 1] >= first[jj]:
            i -= 1
        dmax = max(dmax, jj - i)
    return max(1, min(dmax, 63))


def get_program(dmax):
    key = ("v5", dmax)
    if key not in _prog_cache:
        _prog_cache[key] = _build_program(dmax)
    return _prog_cache[key]


def make_in_maps(embeddings, partition_labels):
    emb = np.asarray(embeddings, dtype=np.float32)
    labels = np.asarray(partition_labels).astype(np.int64)
    perm = np.argsort(labels, kind="stable")
    E_s = emb[perm]
    lab_s = labels[perm]
    lab_f = lab_s.astype(np.float32)

    dmax = _compute_dmax(lab_s)
    E_sT = np.ascontiguousarray(E_s.T).astype(BF16)
    dia = np.exp(np.sum(E_s.astype(np.float64) ** 2, axis=1) / TEMP)

    cls = np.arange(64, dtype=np.int64)
    in_maps = []
    for c in range(NC):
        idx = (np.arange(N) + c * RPC) % N
        ktrot = np.ascontiguousarray(E_sT[:, idx])
        kl = lab_f[idx[:NJJ * 128]].reshape(NJJ, 128).T
        koh = (lab_s[idx[:NJJ * 128]].reshape(NJJ, 128)[:, :, None]
               == cls[None, None, :])
        koh = np.ascontiguousarray(
            koh.transpose(1, 0, 2).reshape(128, NJJ * 64)).astype(BF16)
        qlab_c = lab_f[c * RPC:(c + 1) * RPC]
        qlabb = np.ascontiguousarray(
            np.broadcast_to(qlab_c.astype(BF16)[None, :], (128, RPC)))
        qoh = (lab_s[c * RPC:(c + 1) * RPC][None, :] == cls[:, None])
        qoh2 = np.ascontiguousarray(
            np.vstack([qoh, qoh])).astype(BF16)      # [128, RPC]
        in_maps.append({
            "kt": ktrot,
            "koh": koh,
            "klab": np.ascontiguousarray(kl).astype(np.float32),
            "qlabb": qlabb,
            "qoh": qoh2,
        })
    return in_maps, lab_s, dmax, dia


def combine(results, lab_s, PW, dia):
    A = np.zeros(N, dtype=np.float64)
    P = np.zeros(N, dtype=np.float64)
    for c, r in enumerate(results):
        base = c * RPC
        aqpq = np.asarray(r["aqpq"], dtype=np.float64)
        for b in range(2):
            sl = slice(b * 512, (b + 1) * 512)
            A[base + b * 1024:base + b * 1024 + 512] += aqpq[0, sl]
            A[base + b * 1024 + 512:base + (b + 1) * 1024] += aqpq[1, sl]
            P[base + b * 1024:base + b * 1024 + 512] += aqpq[2, sl]
            P[base + b * 1024 + 512:base + (b + 1) * 1024] += aqpq[3, sl]
        ra2 = np.asarray(r["rowacc2"], dtype=np.float64)
        ra = ra2[:, 0::2] + ra2[:, 1::2]
        pk = np.asarray(r["poskey"], dtype=np.float64)
        for jj in range(NJJ - 1):
            g = (base + jj * 128) % N
            A[g:g + 128] += ra[:, jj]
            if jj < PW:
                P[g:g + 128] += pk[:, jj]
    A -= dia
    P -= dia

    counts = np.bincount(lab_s, minlength=1)
    valid = counts[lab_s] >= 2
    n_valid = int(valid.sum())
    if n_valid == 0:
        return np.float32(0.0)
    loss = np.log(A) - np.log(np.maximum(P, 1e-300))
    return np.float32(loss[valid].sum() / n_valid)


def kernel(embeddings, partition_labels):
    from concourse.bass_utils import run_bass_kernel_spmd

    in_maps, lab_s, dmax, dia = make_in_maps(embeddings, partition_labels)
    nc, PW = get_program(dmax)
    res = run_bass_kernel_spmd(nc, in_maps, list(range(NC)))
    return combine(res.results, lab_s, PW, dia)


# revision 39
# speedup vs baseline: 1.1824x; 1.0070x over previous
"""v5: v4 + head/tail overhaul from trace analysis.

Trace findings on v4 (189.4us):
- scalar (ACT) is the pacer: 100% busy 38us..157us; head has ~14us of
  ACT idle (koh DMA queued behind 2MB of kt tiles -> 8.4us full-pipe
  stall), tail has ~13us after the last exp (fp32 LOW_HIGH ones-fold
  matmuls ~8.5us + serialized drains/DMAs), plus ~16us fixed NEFF
  pre/postamble.

v5 changes:
- DMA order: kq0 split (first 128 cols land first), kq1, koh_a (first
  16 jj), kq2, kq3, klab, qlabb, kts1, koh_b, kts2-4, qoh (bf16).
  Everything lands >=1us before its first consumer.
- LAG taper: 8 for jj<12 shrinking to 2 by jj=18, so the first oh
  matmul enters the PE queue only after koh_a has landed; exp pool
  widened to 10 bufs.
- init_S segmented: one zoh matmul after do_jj(2,4,6,8) instead of a
  2us block that starved ACT at startup.
- rowacc2 [128, 2*(NJJ-1)]: DVE cache-reduce accum -> col 2jj, ACT
  activation accum -> col 2jj+1, memset once, host sums the pair.
  Removes 79 TENSOR_REDUCEs and the racc indirection.
- folds in bf16 inside the steady state: per drained bank, S bank ->
  bf16 sc_b, pm_b = sc_b * qoh (TT bf16 2x), two bf16 ones-matmuls
  (aq/pq) into S rows 0/1 of the dead bank, [2,512] copy to SBUF.
  Replaces the tail's fp32 folds entirely.
- output DMAs chunked/early: rowacc2 cols for jj<64 DMA'd mid-kernel.
"""

import sys

if "/opt/trn_rl_repo" not in sys.path:
    sys.path.insert(0, "/opt/trn_rl_repo")

from collections import deque

import numpy as np
import ml_dtypes

N = 16384
D = 128
NC = 8
RPC = N // NC
QB = RPC // 128
NBLK = N // 128
NJJ = 80
TEMP = 0.5
BF16 = ml_dtypes.bfloat16

# Schraudolph exp in bf16 bits: bits = round(x*(128/ln2) + 128*(127-c)).
# x = psum/TEMP, folded into the multiplier. Row-sum ratio error ~0.1%
# and the ~+0.3% mean bias cancels between pos_sum and all_sum.
SCH_A = (1.0 / TEMP) * 128.0 / float(np.log(2.0))
SCH_B = 128.0 * (127.0 - 0.0436)

_prog_cache = {}


def _seg512(a, b):
    """Split [a, b) at absolute multiples of 512 (matmul ISA max width)."""
    out = []
    while a < b:
        n = min((a // 512 + 1) * 512, b) - a
        out.append((a, a + n))
        a += n
    return out


def _build_program(dmax):
    import concourse.bacc as bacc
    import concourse.tile as tile
    import concourse.mybir as mybir

    dt = mybir.dt
    AF = mybir.ActivationFunctionType
    ALU = mybir.AluOpType

    PW = min(NJJ - 1, QB + dmax)
    KOH_A = 16 * 64  # one-hot cols for jj 0..15

    nc = bacc.Bacc(
        "TRN2",
        target_bir_lowering=False,
        debug=False,
        enable_asserts=False,
        num_devices=NC,
    )

    kt_d = nc.dram_tensor("kt", [D, N], dt.bfloat16, kind="ExternalInput").ap()
    koh_d = nc.dram_tensor("koh", [128, NJJ * 64], dt.bfloat16, kind="ExternalInput").ap()
    klab_d = nc.dram_tensor("klab", [128, NJJ], dt.float32, kind="ExternalInput").ap()
    qlabb_d = nc.dram_tensor("qlabb", [128, RPC], dt.bfloat16, kind="ExternalInput").ap()
    qoh_d = nc.dram_tensor("qoh", [128, RPC], dt.bfloat16, kind="ExternalInput").ap()

    rowacc_d = nc.dram_tensor(
        "rowacc2", [128, 2 * (NJJ - 1)], dt.float32, kind="ExternalOutput").ap()
    poskey_d = nc.dram_tensor("poskey", [128, PW], dt.float32, kind="ExternalOutput").ap()
    # rows: 0=aq_lo 1=aq_hi 2=pq_lo 3=pq_hi; col = b*512 + (c % 512) for
    # S-bank b, where lo = queries [b*1024, b*1024+512), hi = +512
    aqpq_d = nc.dram_tensor("aqpq", [4, RPC // 2], dt.float32, kind="ExternalOutput").ap()

    with tile.TileContext(nc) as tc:
        with (
            tc.tile_pool(name="keys", bufs=1) as keys_pool,
            tc.tile_pool(name="aux", bufs=1) as aux_pool,
            tc.tile_pool(name="ps", bufs=3, space="PSUM") as psum_pool,
            tc.tile_pool(name="sacc", bufs=1, space="PSUM") as sacc_pool,
            tc.tile_pool(name="ex", bufs=10) as exp_pool,
            tc.tile_pool(name="jk", bufs=2) as junk_pool,
            tc.tile_pool(name="fin", bufs=1) as fin_pool,
        ):
            # --- input DMAs, deadline-ordered on the sync HWDGE ring ---
            kq = keys_pool.tile([D, 2048], dt.bfloat16, tag="kq", name="kq")
            koh_s = aux_pool.tile([128, NJJ * 64], dt.bfloat16, tag="koh")
            klab_s = aux_pool.tile([128, NJJ], dt.float32, tag="klab")
            qlabb_s = aux_pool.tile([128, RPC], dt.bfloat16, tag="qlabb")
            qoh_s = aux_pool.tile([128, RPC], dt.bfloat16, tag="qoh")
            kts = [None] + [
                keys_pool.tile([D, 2048], dt.bfloat16, tag=f"kt{j}", name=f"kt{j}")
                for j in range(1, 5)]

            nc.sync.dma_start(kq[:, 0:128], kt_d[:, 0:128])
            nc.sync.dma_start(kq[:, 128:512], kt_d[:, 128:512])
            nc.sync.dma_start(kq[:, 512:1024], kt_d[:, 512:1024])
            nc.sync.dma_start(koh_s[:, 0:KOH_A], koh_d[:, 0:KOH_A])
            nc.sync.dma_start(kq[:, 1024:1536], kt_d[:, 1024:1536])
            nc.sync.dma_start(kq[:, 1536:2048], kt_d[:, 1536:2048])
            nc.sync.dma_start(klab_s[:], klab_d[:])
            nc.sync.dma_start(qlabb_s[:], qlabb_d[:])
            nc.sync.dma_start(kts[1][:], kt_d[:, 2048:4096])
            nc.sync.dma_start(koh_s[:, KOH_A:], koh_d[:, KOH_A:])
            nc.sync.dma_start(kts[2][:], kt_d[:, 4096:6144])
            nc.sync.dma_start(kts[3][:], kt_d[:, 6144:8192])
            nc.sync.dma_start(kts[4][:], kt_d[:, 8192:10240])
            nc.sync.dma_start(qoh_s[:], qoh_d[:])

            def kt_block(b):  # [128, 128] lhsT slice for key block b
                col = b * 128
                if col < 2048:
                    return kq[:, col:col + 128]
                return kts[col // 2048][:, col % 2048:col % 2048 + 128]

            zoh = aux_pool.tile([128, 128], dt.bfloat16, tag="zoh")
            nc.vector.memset(zoh[:], 0.0)
            ones128 = aux_pool.tile([128, 1], dt.bfloat16, tag="ones128")
            nc.vector.memset(ones128[:], 1.0)

            rowacc = fin_pool.tile([128, 2 * (NJJ - 1)], dt.float32, tag="rowacc")
            nc.vector.memset(rowacc[:], 0.0)
            poskey = fin_pool.tile([128, PW], dt.float32, tag="poskey")

            # S packed into 2 PSUM banks [128, 1024]: query col c lives at
            # (rows h*64..h*64+63, col (c//1024)*512 + c%512) with
            # h = (c//512)%2 — frees 2 banks so the exp psum ring gets 3
            # buffers and the PE no longer waits directly on the window's
            # (possibly slow DVE) consumer.
            S = sacc_pool.tile([128, RPC // 2], dt.float32, tag="sacc")

            def init_S_seg(t):
                nc.tensor.matmul(
                    S[:, t * 512:(t + 1) * 512], zoh[:],
                    kq[:, 0:512],
                    start=True, stop=False, skip_group_check=True,
                )

            def oh_mm(jj, lo, a, b, ex, stop):
                for (s0, s1) in _seg512(a, b):
                    th = (s0 // 512) % 2
                    c0 = (s0 // 1024) * 512 + (s0 % 512)
                    nc.tensor.matmul(
                        S[th * 64:(th + 1) * 64, c0:c0 + (s1 - s0)],
                        koh_s[:, jj * 64:(jj + 1) * 64],
                        ex[:, s0 - lo:s1 - lo],
                        start=False, stop=stop,
                        skip_group_check=True,
                        tile_position=(0, th * 64),
                    )

            pending = deque()

            def mode_for(jj):
                # 'b': exact, window-1 rowsum on DVE cache-reduce (early,
                #      poskey band jj; also keeps the diagonal exact)
                # 'A': window-2 exp on DVE (Schraudolph bits; window-1 stays
                #      on ACT with accum so the psum ring never waits on the
                #      DVE), window-2 rowsum via DVE cache-reduce
                # 'a': window-1 exp on DVE (for jj>=64 whose window-1 holds
                #      the extra region and can't use the ACT accum)
                # 'c': exact, BOTH windows' rowsums on ACT accumulators;
                #      frees ~1200ns of DVE per jj
                if jj < 16:
                    return 'b'
                if jj < 20:
                    # both rowsums on ACT: keeps the early DVE backlog
                    # (stt + catch-up cache-reduces) off the critical path
                    return 'c'
                if jj >= 72:
                    # single-window tail jj: offload would serialize on DVE,
                    # 'c' would pollute rowacc with the extra region
                    return 'b'
                if jj >= 64:
                    return 'a'
                return 'a' if jj % 3 != 2 else 'c'

            def service(rec):
                jj, lo, end, acc_lo, oh_end, stt_lo, ex, mode = rec
                # one-hot class sums: extra (d=64) part full weight, main
                # part excludes the d=0 block
                if lo < acc_lo:
                    oh_mm(jj, lo, lo, acc_lo, ex, stop=True)
                if acc_lo < oh_end:
                    oh_mm(jj, lo, acc_lo, oh_end, ex, stop=False)
                # per-key row sum, first window's share on the DVE (the
                # second window accumulated on ACT during exp)
                d1 = 1024 if (lo < 1024 and end > 1024) else end
                if mode == 'A':
                    # window-2 was Schraudolph'd on the DVE: its rowsum too
                    junk = junk_pool.tile([128, 2048], dt.bfloat16, tag="jk", name="jk")
                    nc.vector.tensor_scalar(
                        junk[:, :end - d1], ex[:, d1 - lo:end - lo],
                        1.0, 0.0, ALU.mult, ALU.add,
                        accum_out=rowacc[:, 2 * jj + 1:2 * jj + 2],
                    )
                elif mode != 'c' and acc_lo < d1:
                    junk = junk_pool.tile([128, 2048], dt.bfloat16, tag="jk", name="jk")
                    nc.vector.tensor_scalar(
                        junk[:, :d1 - acc_lo], ex[:, acc_lo - lo:d1 - lo],
                        1.0, 0.0, ALU.mult, ALU.add,
                        accum_out=rowacc[:, 2 * jj:2 * jj + 1],
                    )
                # per-key positive sum over the same-class band
                if stt_lo is not None and stt_lo < end:
                    mk = junk_pool.tile([128, 2048], dt.bfloat16, tag="jk", name="mk")
                    nc.vector.scalar_tensor_tensor(
                        mk[:, :end - stt_lo], qlabb_s[:, stt_lo:end],
                        klab_s[:, jj:jj + 1],
                        ex[:, stt_lo - lo:end - lo],
                        ALU.is_equal, ALU.mult,
                        accum_out=poskey[:, jj:jj + 1],
                    )

            def lag_for(jj):
                if jj >= 77:
                    # drain the pipeline early so the tail services overlap
                    # the last exps instead of running after them
                    return 1
                return max(2, min(8, 8 - (jj - 11) // 2))

            def do_jj(jj):
                main_lo = max(0, jj - 63)
                hi = min(QB - 1, jj)
                lo = (jj - 64 if jj >= 64 else main_lo) * 128
                end = (hi + 1) * 128
                acc_lo = main_lo * 128
                oh_end = min(end, jj * 128) if jj <= QB - 1 else end
                stt_lo = max(acc_lo, (jj - dmax) * 128) if jj <= QB - 1 + dmax else None

                mode = mode_for(jj)
                ex = exp_pool.tile([128, 2048], dt.bfloat16, tag="ex", name="ex")
                k = 0
                w0 = (lo // 1024) * 1024
                while w0 < end:
                    p_lo = max(w0, lo)
                    p_end = min(w0 + 1024, end)
                    if p_lo < p_end:
                        ps = psum_pool.tile([128, 1024], dt.float32, tag="ps", name="ps")
                        for (s0, s1) in _seg512(p_lo, p_end):
                            nc.tensor.matmul(
                                ps[:, s0 - w0:s1 - w0],
                                kt_block(jj), kq[:, s0:s1],
                            )
                        if k == 1 and mode == 'A':
                            # Schraudolph window-2 on the DVE: window-1's
                            # psum consumer stays the (fast) ACT
                            nc.vector.tensor_scalar(
                                ex[:, p_lo - lo:p_end - lo].bitcast(dt.int16),
                                ps[:, p_lo - w0:p_end - w0],
                                SCH_A, SCH_B, ALU.mult, ALU.add,
                            )
                        elif k == 1 and p_lo >= acc_lo:
                            # second window: row-sum for free on the ACT accum
                            nc.scalar.activation(
                                ex[:, p_lo - lo:p_end - lo],
                                ps[:, p_lo - w0:p_end - w0],
                                AF.Exp, scale=1.0 / TEMP,
                                accum_out=rowacc[:, 2 * jj + 1:2 * jj + 2],
                            )
                        elif k == 0 and mode == 'A':
                            # first window: row-sum on the ACT accum
                            nc.scalar.activation(
                                ex[:, p_lo - lo:p_end - lo],
                                ps[:, p_lo - w0:p_end - w0],
                                AF.Exp, scale=1.0 / TEMP,
                                accum_out=rowacc[:, 2 * jj:2 * jj + 1],
                            )
                        elif k == 0 and mode == 'a':
                            # Schraudolph: bf16 bits via DVE fma + int16
                            # round, frees the ACT for other windows
                            nc.vector.tensor_scalar(
                                ex[:, p_lo - lo:p_end - lo].bitcast(dt.int16),
                                ps[:, p_lo - w0:p_end - w0],
                                SCH_A, SCH_B, ALU.mult, ALU.add,
                            )
                        elif k == 0 and mode == 'c':
                            # first window: row-sum on the ACT accum too
                            nc.scalar.activation(
                                ex[:, p_lo - lo:p_end - lo],
                                ps[:, p_lo - w0:p_end - w0],
                                AF.Exp, scale=1.0 / TEMP,
                                accum_out=rowacc[:, 2 * jj:2 * jj + 1],
                            )
                        else:
                            nc.scalar.activation(
                                ex[:, p_lo - lo:p_end - lo],
                                ps[:, p_lo - w0:p_end - w0],
                                AF.Exp, scale=1.0 / TEMP,
                            )
                        k += 1
                    w0 += 1024
                pending.append((jj, lo, end, acc_lo, oh_end, stt_lo, ex, mode))
                while len(pending) > lag_for(jj):
                    service(pending.popleft())

            # per S-bank b (query cols [b*1024,(b+1)*1024)): the low half
            # (rows 0-63) holds q[b*1024 : b*1024+512] class sums, the high
            # half q[b*1024+512 : (b+1)*1024). Each half dies 4 jj apart
            # (after the extra-part oh of jj 67/71/75/79), so fold quarter
            # by quarter: ones64-matmuls staged in the dead half's rows.
            aqpq_s = fin_pool.tile([65, RPC // 2], dt.float32, tag="aqpqs")
            aqpq2_s = fin_pool.tile([65, RPC // 2], dt.float32, tag="aqpq2s")

            def fold_half(b, h):
                sl = slice(b * 512, (b + 1) * 512)
                r = slice(h * 64, (h + 1) * 64)
                qsl = slice(b * 1024 + h * 512, b * 1024 + (h + 1) * 512)
                sc_b = fin_pool.tile([128, 512], dt.bfloat16, tag=f"sc{b}")
                nc.vector.tensor_copy(sc_b[r, :], S[r, sl])
                pm_b = fin_pool.tile([128, 512], dt.bfloat16, tag=f"pm{b}")
                nc.vector.tensor_mul(pm_b[r, :], sc_b[r, :], qoh_s[r, qsl])
                if h == 0:
                    # rows 0-63 dead: stage aq -> row 0, pq -> row 32
                    nc.tensor.matmul(
                        S[0:1, sl], ones128[0:64], sc_b[0:64, :],
                        start=True, stop=True, skip_group_check=True)
                    nc.tensor.matmul(
                        S[32:33, sl], ones128[0:64], pm_b[0:64, :],
                        start=True, stop=True, skip_group_check=True)
                    nc.vector.tensor_copy(aqpq_s[:33, sl], S[0:33, sl])
                else:
                    # whole bank dead (row 0 already copied out): stage
                    # aq -> row 64, pq -> row 0
                    nc.tensor.matmul(
                        S[64:65, sl], ones128[64:128], sc_b[64:128, :],
                        start=True, stop=True, skip_group_check=True)
                    nc.tensor.matmul(
                        S[0:1, sl], ones128[64:128], pm_b[64:128, :],
                        start=True, stop=True, skip_group_check=True)
                    nc.vector.tensor_copy(aqpq2_s[:, sl], S[0:65, sl])

            do_jj(0)
            for jj in range(1, NJJ):
                do_jj(jj)
                if jj in (2, 4):
                    init_S_seg((jj - 2) // 2)
                # quarter-folds as each 64-row half dies: q[0:512) after
                # service(67)@69, q[512:1024) after service(71)@73,
                # q[1024:1536) after service(75)@77
                if jj in (69, 73, 77):
                    q = (jj - 69) // 4
                    fold_half(q // 2, q % 2)
                if jj == 67:
                    # rowacc for jj<64 is final (service(65) ran): ship it
                    nc.sync.dma_start(rowacc_d[:, 0:128], rowacc[:, 0:128])
            while pending:
                service(pending.popleft())
            # ship rowacc/poskey while the last quarter-fold computes
            nc.sync.dma_start(rowacc_d[:, 128:], rowacc[:, 128:])
            nc.sync.dma_start(poskey_d[:], poskey[:])
            fold_half(1, 1)

            nc.sync.dma_start(aqpq_d[0:1, :], aqpq_s[0:1, :])
            nc.sync.dma_start(aqpq_d[1:2, :], aqpq2_s[64:65, :])
            nc.sync.dma_start(aqpq_d[2:3, :], aqpq_s[32:33, :])
            nc.sync.dma_start(aqpq_d[3:4, :], aqpq2_s[0:1, :])

    nc.compile()
    return nc, PW


def _compute_dmax(lab_s):
    first = lab_s.reshape(NBLK, 128)[:, 0]
    last = lab_s.reshape(NBLK, 128)[:, -1]
    dmax = 0
    for jj in range(NBLK):
        i = jj
        while i > 0 and last[i - 1] >= first[jj]:
            i -= 1
        dmax = max(dmax, jj - i)
    return max(1, min(dmax, 63))


def get_program(dmax):
    key = ("v5", dmax)
    if key not in _prog_cache:
        _prog_cache[key] = _build_program(dmax)
    return _prog_cache[key]


def make_in_maps(embeddings, partition_labels):
    emb = np.asarray(embeddings, dtype=np.float32)
    labels = np.asarray(partition_labels).astype(np.int64)
    perm = np.argsort(labels, kind="stable")
    E_s = emb[perm]
    lab_s = labels[perm]
    lab_f = lab_s.astype(np.float32)

    dmax = _compute_dmax(lab_s)
    E_sT = np.ascontiguousarray(E_s.T).astype(BF16)
    dia = np.exp(np.sum(E_s.astype(np.float64) ** 2, axis=1) / TEMP)

    cls = np.arange(64, dtype=np.int64)
    in_maps = []
    for c in range(NC):
        idx = (np.arange(N) + c * RPC) % N
        ktrot = np.ascontiguousarray(E_sT[:, idx])
        kl = lab_f[idx[:NJJ * 128]].reshape(NJJ, 128).T
        koh = (lab_s[idx[:NJJ * 128]].reshape(NJJ, 128)[:, :, None]
               == cls[None, None, :])
        koh = np.ascontiguousarray(
            koh.transpose(1, 0, 2).reshape(128, NJJ * 64)).astype(BF16)
        qlab_c = lab_f[c * RPC:(c + 1) * RPC]
        qlabb = np.ascontiguousarray(
            np.broadcast_to(qlab_c.astype(BF16)[None, :], (128, RPC)))
        qoh = (lab_s[c * RPC:(c + 1) * RPC][None, :] == cls[:, None])
        qoh2 = np.ascontiguousarray(
            np.vstack([qoh, qoh])).astype(BF16)      # [128, RPC]
        in_maps.append({
            "kt": ktrot,
            "koh": koh,
            "klab": np.ascontiguousarray(kl).astype(np.float32),
            "qlabb": qlabb,
            "qoh": qoh2,
        })
    return in_maps, lab_s, dmax, dia


def combine(results, lab_s, PW, dia):
    A = np.zeros(N, dtype=np.float64)
    P = np.zeros(N, dtype=np.float64)
    for c, r in enumerate(results):
        base = c * RPC
        aqpq = np.asarray(r["aqpq"], dtype=np.float64)
        for b in range(2):
            sl = slice(b * 512, (b + 1) * 512)
            A[base + b * 1024:base + b * 1024 + 512] += aqpq[0, sl]
            A[base + b * 1024 + 512:base + (b + 1) * 1024] += aqpq[1, sl]
            P[base + b * 1024:base + b * 1024 + 512] += aqpq[2, sl]
            P[base + b * 1024 + 512:base + (b + 1) * 1024] += aqpq[3, sl]
        ra2 = np.asarray(r["rowacc2"], dtype=np.float64)
        ra = ra2[:, 0::2] + ra2[:, 1::2]
        pk = np.asarray(r["poskey"], dtype=np.float64)
        for jj in range(NJJ - 1):
            g = (base + jj * 128) % N
            A[g:g + 128] += ra[:, jj]
            if jj < PW:
                P[g:g + 128] += pk[:, jj]
    A -= dia
    P -= dia

    counts = np.bincount(lab_s, minlength=1)
    valid = counts[lab_s] >= 2
    n_valid = int(valid.sum())
    if n_valid == 0:
        return np.float32(0.0)
    loss = np.log(A) - np.log(np.maximum(P, 1e-300))
    return np.float32(loss[valid].sum() / n_valid)


def kernel(embeddings, partition_labels):
    from concourse.bass_utils import run_bass_kernel_spmd

    in_maps, lab_s, dmax, dia = make_in_maps(embeddings, partition_labels)
    nc, PW = get_program(dmax)
    res = run_bass_kernel_spmd(nc, in_maps, list(range(NC)))
    return combine(res.results, lab_s, PW, dia)


# revision 40
# speedup vs baseline: 1.1915x; 1.0077x over previous
"""v5: v4 + head/tail overhaul from trace analysis.

Trace findings on v4 (189.4us):
- scalar (ACT) is the pacer: 100% busy 38us..157us; head has ~14us of
  ACT idle (koh DMA queued behind 2MB of kt tiles -> 8.4us full-pipe
  stall), tail has ~13us after the last exp (fp32 LOW_HIGH ones-fold
  matmuls ~8.5us + serialized drains/DMAs), plus ~16us fixed NEFF
  pre/postamble.

v5 changes:
- DMA order: kq0 split (first 128 cols land first), kq1, koh_a (first
  16 jj), kq2, kq3, klab, qlabb, kts1, koh_b, kts2-4, qoh (bf16).
  Everything lands >=1us before its first consumer.
- LAG taper: 8 for jj<12 shrinking to 2 by jj=18, so the first oh
  matmul enters the PE queue only after koh_a has landed; exp pool
  widened to 10 bufs.
- init_S segmented: one zoh matmul after do_jj(2,4,6,8) instead of a
  2us block that starved ACT at startup.
- rowacc2 [128, 2*(NJJ-1)]: DVE cache-reduce accum -> col 2jj, ACT
  activation accum -> col 2jj+1, memset once, host sums the pair.
  Removes 79 TENSOR_REDUCEs and the racc indirection.
- folds in bf16 inside the steady state: per drained bank, S bank ->
  bf16 sc_b, pm_b = sc_b * qoh (TT bf16 2x), two bf16 ones-matmuls
  (aq/pq) into S rows 0/1 of the dead bank, [2,512] copy to SBUF.
  Replaces the tail's fp32 folds entirely.
- output DMAs chunked/early: rowacc2 cols for jj<64 DMA'd mid-kernel.
"""

import sys

if "/opt/trn_rl_repo" not in sys.path:
    sys.path.insert(0, "/opt/trn_rl_repo")

from collections import deque

import numpy as np
import ml_dtypes

N = 16384
D = 128
NC = 8
RPC = N // NC
QB = RPC // 128
NBLK = N // 128
NJJ = 80
TEMP = 0.5
BF16 = ml_dtypes.bfloat16

# Schraudolph exp in bf16 bits: bits = round(x*(128/ln2) + 128*(127-c)).
# x = psum/TEMP, folded into the multiplier. Row-sum ratio error ~0.1%
# and the ~+0.3% mean bias cancels between pos_sum and all_sum.
SCH_A = (1.0 / TEMP) * 128.0 / float(np.log(2.0))
SCH_B = 128.0 * (127.0 - 0.0436)

_prog_cache = {}


def _seg512(a, b):
    """Split [a, b) at absolute multiples of 512 (matmul ISA max width)."""
    out = []
    while a < b:
        n = min((a // 512 + 1) * 512, b) - a
        out.append((a, a + n))
        a += n
    return out


def _build_program(dmax):
    import concourse.bacc as bacc
    import concourse.tile as tile
    import concourse.mybir as mybir

    dt = mybir.dt
    AF = mybir.ActivationFunctionType
    ALU = mybir.AluOpType

    PW = min(NJJ - 1, QB + dmax)
    KOH_A = 16 * 64  # one-hot cols for jj 0..15

    nc = bacc.Bacc(
        "TRN2",
        target_bir_lowering=False,
        debug=False,
        enable_asserts=False,
        num_devices=NC,
    )

    kt_d = nc.dram_tensor("kt", [D, N], dt.bfloat16, kind="ExternalInput").ap()
    koh_d = nc.dram_tensor("koh", [128, NJJ * 64], dt.bfloat16, kind="ExternalInput").ap()
    klab_d = nc.dram_tensor("klab", [128, NJJ], dt.float32, kind="ExternalInput").ap()
    qlabb_d = nc.dram_tensor("qlabb", [128, RPC], dt.bfloat16, kind="ExternalInput").ap()
    qoh_d = nc.dram_tensor("qoh", [128, RPC], dt.bfloat16, kind="ExternalInput").ap()

    rowacc_d = nc.dram_tensor(
        "rowacc2", [128, 2 * (NJJ - 1)], dt.float32, kind="ExternalOutput").ap()
    poskey_d = nc.dram_tensor("poskey", [128, PW], dt.float32, kind="ExternalOutput").ap()
    # rows: 0=aq_lo 1=aq_hi 2=pq_lo 3=pq_hi; col = b*512 + (c % 512) for
    # S-bank b, where lo = queries [b*1024, b*1024+512), hi = +512
    aqpq_d = nc.dram_tensor("aqpq", [4, RPC // 2], dt.float32, kind="ExternalOutput").ap()

    with tile.TileContext(nc) as tc:
        with (
            tc.tile_pool(name="keys", bufs=1) as keys_pool,
            tc.tile_pool(name="aux", bufs=1) as aux_pool,
            tc.tile_pool(name="ps", bufs=3, space="PSUM") as psum_pool,
            tc.tile_pool(name="sacc", bufs=1, space="PSUM") as sacc_pool,
            tc.tile_pool(name="ex", bufs=10) as exp_pool,
            tc.tile_pool(name="jk", bufs=2) as junk_pool,
            tc.tile_pool(name="fin", bufs=1) as fin_pool,
        ):
            # --- input DMAs, deadline-ordered on the sync HWDGE ring ---
            kq = keys_pool.tile([D, 2048], dt.bfloat16, tag="kq", name="kq")
            koh_s = aux_pool.tile([128, NJJ * 64], dt.bfloat16, tag="koh")
            klab_s = aux_pool.tile([128, NJJ], dt.float32, tag="klab")
            qlabb_s = aux_pool.tile([128, RPC], dt.bfloat16, tag="qlabb")
            qoh_s = aux_pool.tile([128, RPC], dt.bfloat16, tag="qoh")
            kts = [None] + [
                keys_pool.tile([D, 2048], dt.bfloat16, tag=f"kt{j}", name=f"kt{j}")
                for j in range(1, 5)]

            nc.sync.dma_start(kq[:, 0:128], kt_d[:, 0:128])
            nc.sync.dma_start(kq[:, 128:512], kt_d[:, 128:512])
            nc.sync.dma_start(kq[:, 512:1024], kt_d[:, 512:1024])
            nc.sync.dma_start(koh_s[:, 0:KOH_A], koh_d[:, 0:KOH_A])
            nc.sync.dma_start(kq[:, 1024:1536], kt_d[:, 1024:1536])
            nc.sync.dma_start(klab_s[:], klab_d[:])
            # qlabb front half early: the first poskey STT sits at the head
            # of the DVE queue and otherwise stalls the cache-reduce stream
            nc.sync.dma_start(qlabb_s[:, 0:1024], qlabb_d[:, 0:1024])
            nc.sync.dma_start(kq[:, 1536:2048], kt_d[:, 1536:2048])
            nc.sync.dma_start(qlabb_s[:, 1024:2048], qlabb_d[:, 1024:2048])
            nc.sync.dma_start(kts[1][:], kt_d[:, 2048:4096])
            nc.sync.dma_start(koh_s[:, KOH_A:], koh_d[:, KOH_A:])
            nc.sync.dma_start(kts[2][:], kt_d[:, 4096:6144])
            nc.sync.dma_start(kts[3][:], kt_d[:, 6144:8192])
            nc.sync.dma_start(kts[4][:], kt_d[:, 8192:10240])
            nc.sync.dma_start(qoh_s[:], qoh_d[:])

            def kt_block(b):  # [128, 128] lhsT slice for key block b
                col = b * 128
                if col < 2048:
                    return kq[:, col:col + 128]
                return kts[col // 2048][:, col % 2048:col % 2048 + 128]

            zoh = aux_pool.tile([128, 128], dt.bfloat16, tag="zoh")
            nc.vector.memset(zoh[:], 0.0)
            ones128 = aux_pool.tile([128, 1], dt.bfloat16, tag="ones128")
            nc.vector.memset(ones128[:], 1.0)

            rowacc = fin_pool.tile([128, 2 * (NJJ - 1)], dt.float32, tag="rowacc")
            nc.vector.memset(rowacc[:], 0.0)
            poskey = fin_pool.tile([128, PW], dt.float32, tag="poskey")

            # S packed into 2 PSUM banks [128, 1024]: query col c lives at
            # (rows h*64..h*64+63, col (c//1024)*512 + c%512) with
            # h = (c//512)%2 — frees 2 banks so the exp psum ring gets 3
            # buffers and the PE no longer waits directly on the window's
            # (possibly slow DVE) consumer.
            S = sacc_pool.tile([128, RPC // 2], dt.float32, tag="sacc")

            def init_S_seg(t):
                nc.tensor.matmul(
                    S[:, t * 512:(t + 1) * 512], zoh[:],
                    kq[:, 0:512],
                    start=True, stop=False, skip_group_check=True,
                )

            def oh_mm(jj, lo, a, b, ex, stop):
                for (s0, s1) in _seg512(a, b):
                    th = (s0 // 512) % 2
                    c0 = (s0 // 1024) * 512 + (s0 % 512)
                    nc.tensor.matmul(
                        S[th * 64:(th + 1) * 64, c0:c0 + (s1 - s0)],
                        koh_s[:, jj * 64:(jj + 1) * 64],
                        ex[:, s0 - lo:s1 - lo],
                        start=False, stop=stop,
                        skip_group_check=True,
                        tile_position=(0, th * 64),
                    )

            pending = deque()

            def mode_for(jj):
                # 'b': exact, window-1 rowsum on DVE cache-reduce (early,
                #      poskey band jj; also keeps the diagonal exact)
                # 'A': window-2 exp on DVE (Schraudolph bits; window-1 stays
                #      on ACT with accum so the psum ring never waits on the
                #      DVE), window-2 rowsum via DVE cache-reduce
                # 'a': window-1 exp on DVE (for jj>=64 whose window-1 holds
                #      the extra region and can't use the ACT accum)
                # 'c': exact, BOTH windows' rowsums on ACT accumulators;
                #      frees ~1200ns of DVE per jj
                if jj < 16:
                    return 'b'
                if jj < 20:
                    # both rowsums on ACT: keeps the early DVE backlog
                    # (stt + catch-up cache-reduces) off the critical path
                    return 'c'
                if jj >= 72:
                    # single-window tail jj: offload would serialize on DVE,
                    # 'c' would pollute rowacc with the extra region
                    return 'b'
                if jj >= 64:
                    return 'a'
                return 'a' if jj % 3 != 2 else 'c'

            def service(rec):
                jj, lo, end, acc_lo, oh_end, stt_lo, ex, mode = rec
                # one-hot class sums: extra (d=64) part full weight, main
                # part excludes the d=0 block
                if lo < acc_lo:
                    oh_mm(jj, lo, lo, acc_lo, ex, stop=True)
                if acc_lo < oh_end:
                    oh_mm(jj, lo, acc_lo, oh_end, ex, stop=False)
                # per-key row sum, first window's share on the DVE (the
                # second window accumulated on ACT during exp)
                d1 = 1024 if (lo < 1024 and end > 1024) else end
                if mode == 'A':
                    # window-2 was Schraudolph'd on the DVE: its rowsum too
                    junk = junk_pool.tile([128, 2048], dt.bfloat16, tag="jk", name="jk")
                    nc.vector.tensor_scalar(
                        junk[:, :end - d1], ex[:, d1 - lo:end - lo],
                        1.0, 0.0, ALU.mult, ALU.add,
                        accum_out=rowacc[:, 2 * jj + 1:2 * jj + 2],
                    )
                elif mode != 'c' and acc_lo < d1:
                    junk = junk_pool.tile([128, 2048], dt.bfloat16, tag="jk", name="jk")
                    nc.vector.tensor_scalar(
                        junk[:, :d1 - acc_lo], ex[:, acc_lo - lo:d1 - lo],
                        1.0, 0.0, ALU.mult, ALU.add,
                        accum_out=rowacc[:, 2 * jj:2 * jj + 1],
                    )
                # per-key positive sum over the same-class band
                if stt_lo is not None and stt_lo < end:
                    mk = junk_pool.tile([128, 2048], dt.bfloat16, tag="jk", name="mk")
                    nc.vector.scalar_tensor_tensor(
                        mk[:, :end - stt_lo], qlabb_s[:, stt_lo:end],
                        klab_s[:, jj:jj + 1],
                        ex[:, stt_lo - lo:end - lo],
                        ALU.is_equal, ALU.mult,
                        accum_out=poskey[:, jj:jj + 1],
                    )

            def lag_for(jj):
                if jj >= 77:
                    # drain the pipeline early so the tail services overlap
                    # the last exps instead of running after them
                    return 1
                return max(2, min(8, 8 - (jj - 11) // 2))

            def do_jj(jj):
                main_lo = max(0, jj - 63)
                hi = min(QB - 1, jj)
                lo = (jj - 64 if jj >= 64 else main_lo) * 128
                end = (hi + 1) * 128
                acc_lo = main_lo * 128
                oh_end = min(end, jj * 128) if jj <= QB - 1 else end
                stt_lo = max(acc_lo, (jj - dmax) * 128) if jj <= QB - 1 + dmax else None

                mode = mode_for(jj)
                ex = exp_pool.tile([128, 2048], dt.bfloat16, tag="ex", name="ex")
                k = 0
                w0 = (lo // 1024) * 1024
                while w0 < end:
                    p_lo = max(w0, lo)
                    p_end = min(w0 + 1024, end)
                    if p_lo < p_end:
                        ps = psum_pool.tile([128, 1024], dt.float32, tag="ps", name="ps")
                        for (s0, s1) in _seg512(p_lo, p_end):
                            nc.tensor.matmul(
                                ps[:, s0 - w0:s1 - w0],
                                kt_block(jj), kq[:, s0:s1],
                            )
                        if k == 1 and mode == 'A':
                            # Schraudolph window-2 on the DVE: window-1's
                            # psum consumer stays the (fast) ACT
                            nc.vector.tensor_scalar(
                                ex[:, p_lo - lo:p_end - lo].bitcast(dt.int16),
                                ps[:, p_lo - w0:p_end - w0],
                                SCH_A, SCH_B, ALU.mult, ALU.add,
                            )
                        elif k == 1 and p_lo >= acc_lo:
                            # second window: row-sum for free on the ACT accum
                            nc.scalar.activation(
                                ex[:, p_lo - lo:p_end - lo],
                                ps[:, p_lo - w0:p_end - w0],
                                AF.Exp, scale=1.0 / TEMP,
                                accum_out=rowacc[:, 2 * jj + 1:2 * jj + 2],
                            )
                        elif k == 0 and mode == 'A':
                            # first window: row-sum on the ACT accum
                            nc.scalar.activation(
                                ex[:, p_lo - lo:p_end - lo],
                                ps[:, p_lo - w0:p_end - w0],
                                AF.Exp, scale=1.0 / TEMP,
                                accum_out=rowacc[:, 2 * jj:2 * jj + 1],
                            )
                        elif k == 0 and mode == 'a':
                            # Schraudolph: bf16 bits via DVE fma + int16
                            # round, frees the ACT for other windows
                            nc.vector.tensor_scalar(
                                ex[:, p_lo - lo:p_end - lo].bitcast(dt.int16),
                                ps[:, p_lo - w0:p_end - w0],
                                SCH_A, SCH_B, ALU.mult, ALU.add,
                            )
                        elif k == 0 and mode == 'c':
                            # first window: row-sum on the ACT accum too
                            nc.scalar.activation(
                                ex[:, p_lo - lo:p_end - lo],
                                ps[:, p_lo - w0:p_end - w0],
                                AF.Exp, scale=1.0 / TEMP,
                                accum_out=rowacc[:, 2 * jj:2 * jj + 1],
                            )
                        else:
                            nc.scalar.activation(
                                ex[:, p_lo - lo:p_end - lo],
                                ps[:, p_lo - w0:p_end - w0],
                                AF.Exp, scale=1.0 / TEMP,
                            )
                        k += 1
                    w0 += 1024
                pending.append((jj, lo, end, acc_lo, oh_end, stt_lo, ex, mode))
                while len(pending) > lag_for(jj):
                    service(pending.popleft())

            # per S-bank b (query cols [b*1024,(b+1)*1024)): the low half
            # (rows 0-63) holds q[b*1024 : b*1024+512] class sums, the high
            # half q[b*1024+512 : (b+1)*1024). Each half dies 4 jj apart
            # (after the extra-part oh of jj 67/71/75/79), so fold quarter
            # by quarter: ones64-matmuls staged in the dead half's rows.
            aqpq_s = fin_pool.tile([65, RPC // 2], dt.float32, tag="aqpqs")
            aqpq2_s = fin_pool.tile([65, RPC // 2], dt.float32, tag="aqpq2s")

            def fold_half(b, h):
                sl = slice(b * 512, (b + 1) * 512)
                r = slice(h * 64, (h + 1) * 64)
                qsl = slice(b * 1024 + h * 512, b * 1024 + (h + 1) * 512)
                sc_b = fin_pool.tile([128, 512], dt.bfloat16, tag=f"sc{b}")
                nc.vector.tensor_copy(sc_b[r, :], S[r, sl])
                pm_b = fin_pool.tile([128, 512], dt.bfloat16, tag=f"pm{b}")
                nc.vector.tensor_mul(pm_b[r, :], sc_b[r, :], qoh_s[r, qsl])
                if h == 0:
                    # rows 0-63 dead: stage aq -> row 0, pq -> row 32
                    nc.tensor.matmul(
                        S[0:1, sl], ones128[0:64], sc_b[0:64, :],
                        start=True, stop=True, skip_group_check=True)
                    nc.tensor.matmul(
                        S[32:33, sl], ones128[0:64], pm_b[0:64, :],
                        start=True, stop=True, skip_group_check=True)
                    nc.vector.tensor_copy(aqpq_s[:33, sl], S[0:33, sl])
                else:
                    # whole bank dead (row 0 already copied out): stage
                    # aq -> row 64, pq -> row 0
                    nc.tensor.matmul(
                        S[64:65, sl], ones128[64:128], sc_b[64:128, :],
                        start=True, stop=True, skip_group_check=True)
                    nc.tensor.matmul(
                        S[0:1, sl], ones128[64:128], pm_b[64:128, :],
                        start=True, stop=True, skip_group_check=True)
                    nc.vector.tensor_copy(aqpq2_s[:, sl], S[0:65, sl])

            do_jj(0)
            for jj in range(1, NJJ):
                do_jj(jj)
                if jj in (2, 4):
                    init_S_seg((jj - 2) // 2)
                # quarter-folds as each 64-row half dies: q[0:512) after
                # service(67)@69, q[512:1024) after service(71)@73,
                # q[1024:1536) after service(75)@77
                if jj in (69, 73, 77):
                    q = (jj - 69) // 4
                    fold_half(q // 2, q % 2)
                if jj == 67:
                    # rowacc for jj<64 is final (service(65) ran): ship it
                    nc.sync.dma_start(rowacc_d[:, 0:128], rowacc[:, 0:128])
            while pending:
                service(pending.popleft())
            # ship rowacc/poskey while the last quarter-fold computes
            nc.sync.dma_start(rowacc_d[:, 128:], rowacc[:, 128:])
            nc.sync.dma_start(poskey_d[:], poskey[:])
            fold_half(1, 1)

            nc.sync.dma_start(aqpq_d[0:1, :], aqpq_s[0:1, :])
            nc.sync.dma_start(aqpq_d[1:2, :], aqpq2_s[64:65, :])
            nc.sync.dma_start(aqpq_d[2:3, :], aqpq_s[32:33, :])
            nc.sync.dma_start(aqpq_d[3:4, :], aqpq2_s[0:1, :])

    nc.compile()
    return nc, PW


def _compute_dmax(lab_s):
    first = lab_s.reshape(NBLK, 128)[:, 0]
    last = lab_s.reshape(NBLK, 128)[:, -1]
    dmax = 0
    for jj in range(NBLK):
        i = jj
        while i > 0 and last[i - 1] >= first[jj]:
            i -= 1
        dmax = max(dmax, jj - i)
    return max(1, min(dmax, 63))


def get_program(dmax):
    key = ("v5", dmax)
    if key not in _prog_cache:
        _prog_cache[key] = _build_program(dmax)
    return _prog_cache[key]


def make_in_maps(embeddings, partition_labels):
    emb = np.asarray(embeddings, dtype=np.float32)
    labels = np.asarray(partition_labels).astype(np.int64)
    perm = np.argsort(labels, kind="stable")
    E_s = emb[perm]
    lab_s = labels[perm]
    lab_f = lab_s.astype(np.float32)

    dmax = _compute_dmax(lab_s)
    E_sT = np.ascontiguousarray(E_s.T).astype(BF16)
    dia = np.exp(np.sum(E_s.astype(np.float64) ** 2, axis=1) / TEMP)

    cls = np.arange(64, dtype=np.int64)
    in_maps = []
    for c in range(NC):
        idx = (np.arange(N) + c * RPC) % N
        ktrot = np.ascontiguousarray(E_sT[:, idx])
        kl = lab_f[idx[:NJJ * 128]].reshape(NJJ, 128).T
        koh = (lab_s[idx[:NJJ * 128]].reshape(NJJ, 128)[:, :, None]
               == cls[None, None, :])
        koh = np.ascontiguousarray(
            koh.transpose(1, 0, 2).reshape(128, NJJ * 64)).astype(BF16)
        qlab_c = lab_f[c * RPC:(c + 1) * RPC]
        qlabb = np.ascontiguousarray(
            np.broadcast_to(qlab_c.astype(BF16)[None, :], (128, RPC)))
        qoh = (lab_s[c * RPC:(c + 1) * RPC][None, :] == cls[:, None])
        qoh2 = np.ascontiguousarray(
            np.vstack([qoh, qoh])).astype(BF16)      # [128, RPC]
        in_maps.append({
            "kt": ktrot,
            "koh": koh,
            "klab": np.ascontiguousarray(kl).astype(np.float32),
            "qlabb": qlabb,
            "qoh": qoh2,
        })
    return in_maps, lab_s, dmax, dia


def combine(results, lab_s, PW, dia):
    A = np.zeros(N, dtype=np.float64)
    P = np.zeros(N, dtype=np.float64)
    for c, r in enumerate(results):
        base = c * RPC
        aqpq = np.asarray(r["aqpq"], dtype=np.float64)
        for b in range(2):
            sl = slice(b * 512, (b + 1) * 512)
            A[base + b * 1024:base + b * 1024 + 512] += aqpq[0, sl]
            A[base + b * 1024 + 512:base + (b + 1) * 1024] += aqpq[1, sl]
            P[base + b * 1024:base + b * 1024 + 512] += aqpq[2, sl]
            P[base + b * 1024 + 512:base + (b + 1) * 1024] += aqpq[3, sl]
        ra2 = np.asarray(r["rowacc2"], dtype=np.float64)
        ra = ra2[:, 0::2] + ra2[:, 1::2]
        pk = np.asarray(r["poskey"], dtype=np.float64)
        for jj in range(NJJ - 1):
            g = (base + jj * 128) % N
            A[g:g + 128] += ra[:, jj]
            if jj < PW:
                P[g:g + 128] += pk[:, jj]
    A -= dia
    P -= dia

    counts = np.bincount(lab_s, minlength=1)
    valid = counts[lab_s] >= 2
    n_valid = int(valid.sum())
    if n_valid == 0:
        return np.float32(0.0)
    loss = np.log(A) - np.log(np.maximum(P, 1e-300))
    return np.float32(loss[valid].sum() / n_valid)


def kernel(embeddings, partition_labels):
    from concourse.bass_utils import run_bass_kernel_spmd

    in_maps, lab_s, dmax, dia = make_in_maps(embeddings, partition_labels)
    nc, PW = get_program(dmax)
    res = run_bass_kernel_spmd(nc, in_maps, list(range(NC)))
    return combine(res.results, lab_s, PW, dia)
